# revision 20
# baseline (speedup 1.0000x reference)
"""Trainium2 Bass kernel for the MultiHeadAttention-variant transformer block.

Math notes (derived from the module semantics):
  - The einsum 'batt,bath->bath' uses only the DIAGONAL of the softmax'd
    attention matrix, so per flat row i the attention output is
    softmax_diag_i * V[i].
  - The raw reshape (B,T,N*H)->(B,N,T,H) makes attention "blocks" couple only
    groups of 128 consecutive tokens (T/N = 1024/8 = 128); a block's 1024
    logical rows are the (chunk c, token t) pairs of those 128 tokens.
  => The whole layer is data-parallel over 128-token groups. We shard the
     4096 flattened tokens as 512 consecutive tokens per core (8 cores), with
     zero cross-core communication.

v2 design (vs the v1 baseline):
  - S is computed Q-STATIONARY with rows enumerated r = c*128 + t, so each
    row-tile of the 1024x1024 block-attention matrix is one out-psum
    [128, 1024] whose FREE axis spans the full softmax denominator.  The
    row sums Z then come for free from the Exp activation's accum_out, and
    the numerators are the diagonals of the c-th 128-col group (identity
    mask * ones-column matmul).  This removes the v1 Z-row matmul chains,
    the [1,512] reciprocals (28us of DVE!), and most broadcast plumbing.
  - All five weight matmuls (QKV, Wo, FFN1, FFN2) run fp8e4m3 DoubleRow.
    Weights are pre-scaled by 16 on the host (values ~0.02*N(0,1) would
    otherwise sit at the bottom of the fp8 range); the scales are folded
    into the exp() activation scale and the two residual descales.
  - LDWEIGHTS is fully hidden behind 512-col DR matmuls (216ns cadence),
    so the tensor-engine floor is the streamed column count.
"""

import sys

sys.path.insert(0, "/opt/trn_rl_repo")

import numpy as np
import ml_dtypes

import concourse.bass as bass
import concourse.mybir as mybir
import concourse.tile as tile
from concourse import bacc, bass_utils

F32 = mybir.dt.float32
BF16 = mybir.dt.bfloat16
F8 = mybir.dt.float8e4
AF = mybir.ActivationFunctionType
ALU = mybir.AluOpType
AX = mybir.AxisListType
DR = mybir.MatmulPerfMode.DoubleRow

H = 512
NH = 8
B = 4
T = 1024
TOK = B * T
NCORES = 8
TPC = TOK // NCORES  # 512 tokens per core
NBLK = TPC // 128  # 4 attention blocks per core
SCALE = float(1.0 / np.sqrt(H))
LN_EPS = 1e-5

_BF = ml_dtypes.bfloat16
_F8 = ml_dtypes.float8_e4m3

WS = 16.0  # host pre-scale on every weight matrix (fp8 range)
DS = 64.0  # extra scale on the softmax-diag D (fp8 range of hhT)
ESC = SCALE / (WS * WS)  # exp() input scale: undo Wq*16 * Wk*16
ODESC = 1.0 / (WS * WS * DS)  # Wo-path descale: V*16, Wo*16, D*64
FDESC = 1.0 / WS  # FFN descales


def _emit(nc, tc, d):
    """Emit the per-core program. d: dict of DRAM APs."""
    P = tc.alloc_tile_pool(name="persist", bufs=1)
    PW = tc.alloc_tile_pool(name="wpool", bufs=12)
    SCR = tc.alloc_tile_pool(name="scr", bufs=4)
    ST = tc.alloc_tile_pool(name="stats", bufs=4)
    PS_S = tc.alloc_tile_pool(name="ps_s", bufs=2, space="PSUM")
    PS_A = tc.alloc_tile_pool(name="ps_a", bufs=2, space="PSUM")
    PS_B = tc.alloc_tile_pool(name="ps_b", bufs=2, space="PSUM")

    # ---- persistent tiles
    xT = P.tile([128, 4 * 512], F8, name="xT")  # x^T, 4 h-chunks
    identf = P.tile([128, 128], F32, name="identf")
    identb = P.tile([128, 128], BF16, name="identb")
    ones_c = P.tile([128, 1], BF16, name="ones_c")  # rhs for diag-col matmul
    ones128 = P.tile([128, 128], BF16, name="ones128")  # bcast lhsT rows
    vrow = P.tile([1, 4 * 512], BF16, name="vrow")  # g1,vsum,g2,b2 rows
    bf1c = P.tile([128, 16], F32, name="bf1c")  # bf1 column-major
    gb = P.tile([128, 4 * 512], F32, name="gb")  # bcast g1,vsum,g2,b2
    Bc = P.tile([128, 8 * 512], BF16, name="Bc")  # D*DS bcast, [p, c*512+t]
    Zt = P.tile([128, 32], F32, name="Zt")  # softmax denoms, col a*8+c
    epsc = P.tile([128, 1], F32, name="epsc")
    xrs = P.tile([128, 4 * 512], F32, name="xrs")  # residual x, [p, mt*512+h]
    sel8 = P.tile([128, 8 * 128], BF16, name="sel8")  # sel8[k,c*128+p]=(k==c)
    QT = P.tile([128, 32 * 512], F8, name="QT")
    KT = P.tile([128, 32 * 512], F8, name="KT")
    hhT = P.tile([128, 32 * 512], F8, name="hhT")
    xcr = P.tile([128, 4 * 512], F32, name="xcr")  # LN1 core out (no g/b)
    hh1T = P.tile([128, 4 * 512], BF16, name="hh1T")  # xcr^T for FFN1
    hh1r = P.tile([128, 4 * 512], F32, name="hh1r")  # true hh1 residual
    a1T = P.tile([128, 16 * 512], BF16, name="a1T")  # relu acts, [p, fc*512+t]
    wos = P.tile([128, 16 * 1024], F8, name="wos")  # Wo DR-packed
    wf1s = P.tile([128, 4 * 2048], BF16, name="wf1s")  # [p, hc*2048+f]
    wf2s = P.tile([128, 16 * 512], BF16, name="wf2s")  # [p, fc*512+h]

    # ---- input DMAs.  sync ring: critical path (x, ident, wq/wk/wv stream).
    # scalar ring: everything needed later (vecs, bf1, xr, wo, wf1, wf2).
    nc.sync.dma_start(
        xT.rearrange("p (hc t) -> p hc t", hc=4),
        d["xT"].rearrange("(hc p) t -> p hc t", p=128))
    nc.sync.dma_start(identf[:], d["ident"][:])
    nc.scalar.dma_start(vrow[0:1, :], d["vecs"].rearrange("v h -> (v h)")[None, :])
    nc.scalar.dma_start(bf1c[:], d["bf1"].rearrange("(m p) -> p m", p=128))
    nc.scalar.dma_start(
        xrs.rearrange("p (mt h) -> p mt h", mt=4),
        d["xr"].rearrange("(mt p) h -> p mt h", p=128))
    nc.scalar.dma_start(
        wos.rearrange("p (i f) -> p i f", i=16),
        d["wo"].rearrange("(i p) f -> p i f", p=128))
    nc.scalar.dma_start(
        wf1s.rearrange("p (hc f) -> p hc f", hc=4),
        d["wf1"].rearrange("(hc p) f -> p hc f", p=128))
    nc.scalar.dma_start(
        wf2s.rearrange("p (fc h) -> p fc h", fc=16),
        d["wf2"].rearrange("(fc p) h -> p fc h", p=128))

    nc.vector.memset(ones_c[:], 1.0)
    nc.vector.memset(ones128[:], 1.0)
    nc.vector.memset(epsc[:], LN_EPS)
    nc.scalar.dma_start(sel8[0:8, :], d["sel8"][:])
    nc.vector.tensor_copy(identb[:], identf[:])

    # ---- projections (fp8 DoubleRow): dst^T = W^T @ x^T.  Weight DRAM is
    # pre-packed tile-major: tile (pair, q) rows,
    # [p, j*1024+m] = W[(2*pair+j)*128+p, q*1024+m].
    xTp = [xT[:, pp * 1024:(pp + 1) * 1024].rearrange("p (j t) -> p j t", j=2)
           for pp in range(2)]

    def proj(wname, evac):
        wsrc = d[wname].rearrange("(t p) f -> t p f", p=128)
        wt = {}
        for q in range(4):
            for pp in range(2):
                w = PW.tile([128, 2048], F8, name=f"w_{wname}{q}{pp}", tag="w")
                nc.sync.dma_start(w[:], wsrc[pp * 4 + q])
                wt[(q, pp)] = w
        for m in range(32):
            q, mq = m // 8, m % 8
            ps = PS_A.tile([128, 512], F32, name="ps_proj", tag="acc")
            for pp in range(2):
                lhsT = wt[(q, pp)].rearrange(
                    "p (j m) -> p j m", j=2)[:, :, mq * 128:(mq + 1) * 128]
                nc.tensor.matmul(ps[:], lhsT=lhsT, rhs=xTp[pp],
                                 start=(pp == 0), stop=(pp == 1),
                                 perf_mode=DR)
            evac(m, ps)

    def evac_alt(dst):
        def f(m, ps):
            sl = dst[:, m * 512:(m + 1) * 512]
            if m % 2 == 0:
                nc.vector.tensor_copy(sl, ps[:])
            else:
                nc.scalar.copy(sl, ps[:])
        return f

    proj("wq", evac_alt(QT))
    proj("wk", evac_alt(KT))

    # ---- gamma/beta broadcast rows -> gb
    for i in range(4):
        psg = PS_A.tile([128, 512], F32, name="psg", tag="acc")
        nc.tensor.matmul(psg[:], lhsT=ones128[0:1, :],
                         rhs=vrow[0:1, i * 512:(i + 1) * 512],
                         start=True, stop=True, tile_position=(0, 0))
        nc.scalar.copy(gb[:, i * 512:(i + 1) * 512], psg[:])

    # ---- attention: Q-stationary S tiles, Z via exp-accum, diag numerators.
    # Row/col enumeration within a block: r = c*128 + t_local.
    QT4 = QT.rearrange("p (c hc t) -> p c hc t", c=8, hc=4)
    KT4 = KT.rearrange("p (c hc t) -> p c hc t", c=8, hc=4)
    Bc3 = Bc.rearrange("p (c t) -> p c t", c=8)

    for a in range(NBLK):
        ts = slice(a * 128, (a + 1) * 128)
        msks = []
        for c in range(8):
            ps = PS_S.tile([128, 1024], F32, name="ps_s", tag="S")
            for half in range(2):
                for pp in range(2):
                    lhsT = QT4[:, c, 2 * pp:2 * pp + 2, ts]
                    rhs = KT4[:, 4 * half:4 * half + 4,
                              2 * pp:2 * pp + 2, ts].transpose([0, 2, 1, 3])
                    nc.tensor.matmul(
                        ps[:, half * 512:(half + 1) * 512],
                        lhsT=lhsT, rhs=rhs,
                        start=(pp == 0), stop=(pp == 1), perf_mode=DR)
            junk = SCR.tile([128, 1024], BF16, name="junk", tag="junk", bufs=3)
            nc.scalar.activation(junk[:], ps[:], AF.Exp, scale=ESC,
                                 accum_out=Zt[:, a * 8 + c:a * 8 + c + 1])
            msk = SCR.tile([128, 128], BF16, name="msk", tag="msk", bufs=4)
            nc.vector.tensor_mul(msk[:], junk[:, c * 128:(c + 1) * 128],
                                 identb[:])
            msks.append(msk)
        nmr = PS_B.tile([128, 8], F32, name="nmr", tag="sm")
        for c in range(8):
            nc.tensor.matmul(nmr[:, c:c + 1], lhsT=msks[c][:], rhs=ones_c[:],
                             start=True, stop=True, skip_group_check=True)
        # D = numer * DS/Z -> transpose to rows 0..7 -> selector-matmul
        # broadcast (all reads at base partition 0; offset bases hang the HW)
        zrt = SCR.tile([128, 8], F32, name="zrt", tag="zrt", bufs=2)
        nc.vector.reciprocal(zrt[:], Zt[:, a * 8:a * 8 + 8])
        nc.vector.tensor_scalar_mul(zrt[:], zrt[:], DS)
        dc = SCR.tile([128, 8], BF16, name="dc", tag="dc", bufs=2)
        nc.vector.tensor_mul(dc[:], nmr[:], zrt[:])
        dt_ps = PS_B.tile([128, 128], BF16, name="dt_ps", tag="sm")
        nc.tensor.transpose(dt_ps[0:8, :], dc[:], identb[:])
        dt_sb = SCR.tile([128, 128], BF16, name="dt_sb", tag="dts", bufs=2)
        nc.vector.tensor_copy(dt_sb[0:8, :], dt_ps[0:8, :])
        bc_ps = PS_S.tile([128, 1024], F32, name="bc_ps", tag="S")
        for c in range(8):
            nc.tensor.matmul(bc_ps[:, c * 128:(c + 1) * 128],
                             lhsT=sel8[0:8, c * 128:(c + 1) * 128],
                             rhs=dt_sb[0:8, :],
                             start=True, stop=True, skip_group_check=True)
        nc.vector.tensor_copy(Bc3[:, :, ts],
                              bc_ps.rearrange("p (c t) -> p c t", c=8))

    # ---- V projection fused with diag-softmax scaling -> hhT,
    # then attn out = hhT @ Wo ; + residual ; LN1 core
    hhT3 = hhT.rearrange("p (m t) -> p m t", t=512)
    proj("wv", lambda m, ps: nc.vector.tensor_mul(
        hhT[:, m * 512:(m + 1) * 512], ps[:],
        Bc[:, (m // 4) * 512:(m // 4 + 1) * 512]))

    def ln_core(v_ap, out_ap):
        nmu = ST.tile([128, 1], F32, name="nmu", tag="nmu")
        nc.vector.reduce_sum(nmu[:], v_ap, axis=AX.X)
        nc.vector.tensor_scalar_mul(nmu[:], nmu[:], -1.0 / H)
        ssq = ST.tile([128, 1], F32, name="ssq", tag="ssq")
        junkf = SCR.tile([128, 512], BF16, name="junkf", tag="junkf", bufs=2)
        nc.scalar.activation(junkf[:], v_ap, AF.Square, bias=nmu[:],
                             accum_out=ssq[:])
        sd = ST.tile([128, 1], F32, name="sd", tag="sd")
        nc.scalar.activation(sd[:], ssq[:], AF.Sqrt, scale=1.0 / H,
                             bias=epsc[:])
        rs = ST.tile([128, 1], F32, name="rs", tag="rs")
        nc.vector.reciprocal(rs[:], sd[:])
        nc.vector.tensor_scalar(out_ap, v_ap, nmu[:], rs[:],
                                op0=ALU.add, op1=ALU.mult)

    def ln(v_ap, gofs, bofs, out_ap):
        ln_core(v_ap, out_ap)
        nc.vector.tensor_mul(out_ap, out_ap, gb[:, gofs * 512:(gofs + 1) * 512])
        nc.vector.tensor_add(out_ap, out_ap, gb[:, bofs * 512:(bofs + 1) * 512])

    wos3 = wos.rearrange("p (i j h) -> p i j h", i=16, j=2)
    for mt in range(4):
        ps_o = PS_A.tile([128, 512], F32, name="ps_o", tag="acc")
        for i in range(16):
            nc.tensor.matmul(
                ps_o[:], lhsT=hhT3[:, 2 * i:2 * i + 2, mt * 128:(mt + 1) * 128],
                rhs=wos3[:, i], start=(i == 0), stop=(i == 15), perf_mode=DR)
        v1 = SCR.tile([128, 512], F32, name="v1", tag="xr")
        nc.vector.scalar_tensor_tensor(
            out=v1[:], in0=ps_o[:], scalar=ODESC,
            in1=xrs[:, mt * 512:(mt + 1) * 512], op0=ALU.mult, op1=ALU.add)
        ln_core(v1[:], xcr[:, mt * 512:(mt + 1) * 512])

    # ---- transpose xcr -> hh1T (fp8) for the FFN (g1/b1 folded into Wf1);
    # hh1r = xcr*g1 + (beta1+bf2) computed off the critical path
    for mt in range(4):
        for jj in range(4):
            tp = PS_B.tile([128, 128], F32, name="tp_h", tag="sm")
            nc.tensor.transpose(
                tp[:], xcr[:, mt * 512 + jj * 128:mt * 512 + jj * 128 + 128],
                identf[:])
            nc.vector.tensor_copy(
                hh1T[:, jj * 512 + mt * 128:jj * 512 + mt * 128 + 128], tp[:])
    for mt in range(4):
        sl = slice(mt * 512, (mt + 1) * 512)
        nc.vector.tensor_mul(hh1r[:, sl], xcr[:, sl], gb[:, 0:512])
        nc.vector.tensor_add(hh1r[:, sl], hh1r[:, sl], gb[:, 512:1024])

    # ---- FFN1 (bf16): a1T = relu(Wf1'^T @ hh1T + bf1')
    for mf in range(16):
        ps = PS_A.tile([128, 512], F32, name="ps_f1", tag="acc")
        for hc in range(4):
            nc.tensor.matmul(
                ps[:], lhsT=wf1s[:, hc * 2048 + mf * 128:hc * 2048 + mf * 128 + 128],
                rhs=hh1T[:, hc * 512:(hc + 1) * 512],
                start=(hc == 0), stop=(hc == 3))
        nc.scalar.activation(a1T[:, mf * 512:(mf + 1) * 512], ps[:], AF.Relu,
                             bias=bf1c[:, mf:mf + 1])

    # ---- FFN2 (bf16) + residual + LN2 -> out
    for mt in range(4):
        ps = PS_A.tile([128, 512], F32, name="ps_f2", tag="acc")
        for fc in range(16):
            nc.tensor.matmul(
                ps[:],
                lhsT=a1T[:, fc * 512 + mt * 128:fc * 512 + mt * 128 + 128],
                rhs=wf2s[:, fc * 512:(fc + 1) * 512],
                start=(fc == 0), stop=(fc == 15))
        s2 = SCR.tile([128, 512], F32, name="s2", tag="xr")
        nc.vector.tensor_add(s2[:], ps[:], hh1r[:, mt * 512:(mt + 1) * 512])
        outt = SCR.tile([128, 512], F32, name="outt", tag="xr")
        ln(s2[:], 2, 3, outt[:])
        nc.sync.dma_start(d["out"][mt * 128:(mt + 1) * 128, :], outt[:])

    for pool in (PS_B, PS_A, PS_S, ST, SCR, PW, P):
        pool.release()


def build(loop_n=None):
    nc = bacc.Bacc("TRN2", target_bir_lowering=False)
    d = {
        "xT": nc.dram_tensor("xT", (TPC, H), F8, kind="ExternalInput").ap(),
        "xr": nc.dram_tensor("xr", (TPC, H), F32, kind="ExternalInput").ap(),
        "wq": nc.dram_tensor("wq", (1024, 2048), F8, kind="ExternalInput").ap(),
        "wk": nc.dram_tensor("wk", (1024, 2048), F8, kind="ExternalInput").ap(),
        "wv": nc.dram_tensor("wv", (1024, 2048), F8, kind="ExternalInput").ap(),
        "wo": nc.dram_tensor("wo", (2048, 1024), F8, kind="ExternalInput").ap(),
        "wf1": nc.dram_tensor("wf1", (H, 4 * H), BF16, kind="ExternalInput").ap(),
        "wf2": nc.dram_tensor("wf2", (4 * H, H), BF16, kind="ExternalInput").ap(),
        "bf1": nc.dram_tensor("bf1", (4 * H,), F32, kind="ExternalInput").ap(),
        "vecs": nc.dram_tensor("vecs", (4, H), BF16, kind="ExternalInput").ap(),
        "ident": nc.dram_tensor("ident", (128, 128), F32,
                                kind="ExternalInput").ap(),
        "sel8": nc.dram_tensor("sel8", (8, 8 * 128), BF16,
                               kind="ExternalInput").ap(),
        "out": nc.dram_tensor("out", (TPC, H), F32, kind="ExternalOutput").ap(),
    }
    with tile.TileContext(nc) as tc:
        if loop_n is None:
            _emit(nc, tc, d)
        else:
            with tc.For_i(0, loop_n, 1):
                _emit(nc, tc, d)
    nc.finalize()
    return nc


def _pack_w(W):
    # tile-major fp8 packing for DoubleRow projections:
    # tile (pair, q): [p, j*1024+m] = W[(2*pair+j)*128+p, q*1024+m]
    W5 = np.asarray(W, np.float32).reshape(2, 2, 128, 4, 1024)
    return np.ascontiguousarray(
        W5.transpose(0, 3, 2, 1, 4).reshape(8 * 128, 2048)).astype(_F8)


def _pack_wo(W):
    # tile i: [p, j*512+h] = Wo[(2*i+j)*128+p, h]
    W4 = np.asarray(W, np.float32).reshape(16, 2, 128, 512)
    return np.ascontiguousarray(
        W4.transpose(0, 2, 1, 3).reshape(16 * 128, 1024)).astype(_F8)


def make_in_maps(inputs):
    xf = np.ascontiguousarray(
        np.asarray(inputs["x"], np.float32).reshape(TOK, H))
    g1 = np.asarray(inputs["g1"], np.float32)
    wf1 = np.asarray(inputs["Wf1"], np.float32)
    shared = {
        "wq": _pack_w(WS * np.asarray(inputs["Wq"], np.float32)),
        "wk": _pack_w(WS * np.asarray(inputs["Wk"], np.float32)),
        "wv": _pack_w(WS * np.asarray(inputs["Wv"], np.float32)),
        "wo": _pack_wo(WS * np.asarray(inputs["Wo"], np.float32)),
        "wf1": (g1[:, None] * wf1).astype(_BF),
        "wf2": np.asarray(inputs["Wf2"], np.float32).astype(_BF),
        "bf1": (np.asarray(inputs["bf1"], np.float32)
                + np.asarray(inputs["beta1"], np.float32) @ wf1),
        "vecs": np.ascontiguousarray(np.stack([
            g1,
            np.asarray(inputs["beta1"], np.float32)
            + np.asarray(inputs["bf2"], np.float32),
            np.asarray(inputs["g2"], np.float32),
            np.asarray(inputs["beta2"], np.float32)]).astype(_BF)),
        "ident": np.eye(128, dtype=np.float32),
        "sel8": np.ascontiguousarray(
            np.kron(np.eye(8, dtype=np.float32), np.ones((1, 128), np.float32))
            .astype(_BF)),
    }
    in_maps = []
    for c in range(NCORES):
        xs = xf[c * TPC:(c + 1) * TPC]
        m = dict(shared)
        m["xT"] = np.ascontiguousarray(xs.T).astype(_F8)
        m["xr"] = np.ascontiguousarray(xs)
        in_maps.append(m)
    return in_maps


_nc_cache = None


def _get_nc():
    global _nc_cache
    if _nc_cache is None:
        _nc_cache = build()
    return _nc_cache


def kernel(**inputs):
    nc = _get_nc()
    in_maps = make_in_maps(inputs)
    res = bass_utils.run_bass_kernel_spmd(nc, in_maps,
                                          core_ids=list(range(NCORES)))
    out = np.concatenate([r["out"] for r in res.results], axis=0)
    return out.reshape(B, T, H)


if __name__ == "__main__":
    nc = build()
    n_inst = sum(len(bb.instructions) for bb in nc.main_func.blocks)
    print("built OK; instructions:", n_inst)


# revision 26
# speedup vs baseline: 1.1618x; 1.1618x over previous
"""Trainium2 Bass kernel for the MultiHeadAttention-variant transformer block.

Math notes (derived from the module semantics):
  - The einsum 'batt,bath->bath' uses only the DIAGONAL of the softmax'd
    attention matrix, so per flat row i the attention output is
    softmax_diag_i * V[i].
  - The raw reshape (B,T,N*H)->(B,N,T,H) makes attention "blocks" couple only
    groups of 128 consecutive tokens (T/N = 1024/8 = 128); a block's 1024
    logical rows are the (chunk c, token t) pairs of those 128 tokens.
  => The whole layer is data-parallel over 128-token groups. We shard the
     4096 flattened tokens as 512 consecutive tokens per core (8 cores), with
     zero cross-core communication.

v2 design (vs the v1 baseline):
  - S is computed Q-STATIONARY with rows enumerated r = c*128 + t, so each
    row-tile of the 1024x1024 block-attention matrix is one out-psum
    [128, 1024] whose FREE axis spans the full softmax denominator.  The
    row sums Z then come for free from the Exp activation's accum_out, and
    the numerators are the diagonals of the c-th 128-col group (identity
    mask * ones-column matmul).  This removes the v1 Z-row matmul chains,
    the [1,512] reciprocals (28us of DVE!), and most broadcast plumbing.
  - All five weight matmuls (QKV, Wo, FFN1, FFN2) run fp8e4m3 DoubleRow.
    Weights are pre-scaled by 16 on the host (values ~0.02*N(0,1) would
    otherwise sit at the bottom of the fp8 range); the scales are folded
    into the exp() activation scale and the two residual descales.
  - LDWEIGHTS is fully hidden behind 512-col DR matmuls (216ns cadence),
    so the tensor-engine floor is the streamed column count.
"""

import sys

sys.path.insert(0, "/opt/trn_rl_repo")

import numpy as np
import ml_dtypes

import concourse.bass as bass
import concourse.mybir as mybir
import concourse.tile as tile
from concourse import bacc, bass_utils

F32 = mybir.dt.float32
BF16 = mybir.dt.bfloat16
F8 = mybir.dt.float8e4
AF = mybir.ActivationFunctionType
ALU = mybir.AluOpType
AX = mybir.AxisListType
DR = mybir.MatmulPerfMode.DoubleRow

H = 512
NH = 8
B = 4
T = 1024
TOK = B * T
NCORES = 8
TPC = TOK // NCORES  # 512 tokens per core
NBLK = TPC // 128  # 4 attention blocks per core
SCALE = float(1.0 / np.sqrt(H))
LN_EPS = 1e-5

_BF = ml_dtypes.bfloat16
_F8 = ml_dtypes.float8_e4m3

WS = 16.0  # host pre-scale on every weight matrix (fp8 range)
DS = 64.0  # extra scale on the softmax-diag D (fp8 range of hhT)
ESC = SCALE / (WS * WS)  # exp() input scale: undo Wq*16 * Wk*16
ODESC = 1.0 / (WS * WS * DS)  # Wo-path descale: V*16, Wo*16, D*64
FDESC = 1.0 / WS  # FFN descales


def _emit(nc, tc, d):
    """Emit the per-core program. d: dict of DRAM APs."""
    P = tc.alloc_tile_pool(name="persist", bufs=1)
    PW = tc.alloc_tile_pool(name="wpool", bufs=11)
    SCR = tc.alloc_tile_pool(name="scr", bufs=4)
    ST = tc.alloc_tile_pool(name="stats", bufs=4)
    PS_S = tc.alloc_tile_pool(name="ps_s", bufs=2, space="PSUM")
    PS_A = tc.alloc_tile_pool(name="ps_a", bufs=4, space="PSUM")

    # ---- persistent tiles
    xT = P.tile([128, 4 * 512], F8, name="xT")  # x^T, 4 h-chunks
    identf = P.tile([128, 128], F32, name="identf")
    identb = P.tile([128, 128], BF16, name="identb")
    ones_c = P.tile([128, 1], BF16, name="ones_c")  # rhs for diag-col matmul
    ones128 = P.tile([128, 128], BF16, name="ones128")  # bcast lhsT rows
    vrow = P.tile([1, 4 * 512], BF16, name="vrow")  # g1,vsum,g2,b2 rows
    bf1c = P.tile([128, 16], F32, name="bf1c")  # bf1 column-major
    gb = P.tile([128, 4 * 512], F32, name="gb")  # bcast g1,vsum,g2,b2
    Bc = P.tile([128, 8 * 512], BF16, name="Bc")  # D*DS bcast, [p, c*512+t]
    Zt = P.tile([128, 32], F32, name="Zt")  # softmax denoms, col a*8+c
    epsc = P.tile([128, 1], F32, name="epsc")
    xrs = P.tile([128, 4 * 512], F32, name="xrs")  # residual x, [p, mt*512+h]
    sel8 = P.tile([128, 8 * 128], BF16, name="sel8")  # sel8[k,c*128+p]=(k==c)
    QT = P.tile([128, 32 * 512], F8, name="QT")
    KT = P.tile([128, 32 * 512], F8, name="KT")
    hhT = P.tile([128, 32 * 512], F8, name="hhT")
    xcr = P.tile([128, 4 * 512], F32, name="xcr")  # LN1 core out (no g/b)
    hh1T = P.tile([128, 4 * 512], BF16, name="hh1T")  # xcr^T for FFN1
    hh1r = P.tile([128, 4 * 512], F32, name="hh1r")  # true hh1 residual
    a1T = P.tile([128, 16 * 512], BF16, name="a1T")  # relu acts, [p, fc*512+t]
    wos = P.tile([128, 16 * 1024], F8, name="wos")  # Wo DR-packed
    wf1s = P.tile([128, 4 * 2048], BF16, name="wf1s")  # [p, hc*2048+f]
    wf2s = P.tile([128, 16 * 512], BF16, name="wf2s")  # [p, fc*512+h]

    # ---- input DMAs.  sync ring: critical path (x, ident, wq/wk/wv stream).
    # scalar ring: everything needed later (vecs, bf1, xr, wo, wf1, wf2).
    nc.sync.dma_start(
        xT.rearrange("p (hc t) -> p hc t", hc=4),
        d["xT"].rearrange("(hc p) t -> p hc t", p=128))
    nc.sync.dma_start(identf[:], d["ident"][:])
    nc.scalar.dma_start(vrow[0:1, :], d["vecs"].rearrange("v h -> (v h)")[None, :])
    nc.scalar.dma_start(bf1c[:], d["bf1"].rearrange("(m p) -> p m", p=128))
    # big late-deadline weights go on the (otherwise idle) gpsimd SWDGE ring
    # so they don't block the scalar engine during phase-A evacuations
    nc.gpsimd.dma_start(
        xrs.rearrange("p (mt h) -> p mt h", mt=4),
        d["xr"].rearrange("(mt p) h -> p mt h", p=128))
    nc.gpsimd.dma_start(
        wos.rearrange("p (i f) -> p i f", i=16),
        d["wo"].rearrange("(i p) f -> p i f", p=128))
    nc.gpsimd.dma_start(
        wf1s.rearrange("p (hc f) -> p hc f", hc=4),
        d["wf1"].rearrange("(hc p) f -> p hc f", p=128))
    nc.gpsimd.dma_start(
        wf2s.rearrange("p (fc h) -> p fc h", fc=16),
        d["wf2"].rearrange("(fc p) h -> p fc h", p=128))

    nc.vector.memset(ones_c[:], 1.0)
    nc.vector.memset(ones128[:], 1.0)
    nc.vector.memset(epsc[:], LN_EPS)
    nc.scalar.dma_start(sel8[0:8, :], d["sel8"][:])
    nc.vector.tensor_copy(identb[:], identf[:])

    # ---- projections (fp8 DoubleRow): dst^T = W^T @ x^T.  Weight DRAM is
    # pre-packed tile-major: tile (pair, q) rows,
    # [p, j*1024+m] = W[(2*pair+j)*128+p, q*1024+m].
    xTp = [xT[:, pp * 1024:(pp + 1) * 1024].rearrange("p (j t) -> p j t", j=2)
           for pp in range(2)]

    def proj(wname, evac):
        wsrc = d[wname].rearrange("(t p) f -> t p f", p=128)
        wt = {}
        for q in range(4):
            for pp in range(2):
                w = PW.tile([128, 2048], F8, name=f"w_{wname}{q}{pp}", tag="w")
                nc.sync.dma_start(w[:], wsrc[pp * 4 + q])
                wt[(q, pp)] = w
        for m in range(32):
            q, mq = m // 8, m % 8
            ps = PS_A.tile([128, 512], F32, name="ps_proj", tag="acc")
            for pp in range(2):
                lhsT = wt[(q, pp)].rearrange(
                    "p (j m) -> p j m", j=2)[:, :, mq * 128:(mq + 1) * 128]
                nc.tensor.matmul(ps[:], lhsT=lhsT, rhs=xTp[pp],
                                 start=(pp == 0), stop=(pp == 1),
                                 perf_mode=DR)
            evac(m, ps)

    def evac_alt(dst):
        def f(m, ps):
            sl = dst[:, m * 512:(m + 1) * 512]
            if m % 2 == 0:
                nc.vector.tensor_copy(sl, ps[:])
            else:
                nc.scalar.copy(sl, ps[:])
        return f

    proj("wq", evac_alt(QT))
    proj("wk", evac_alt(KT))

    # ---- gamma/beta broadcast rows -> gb
    for i in range(4):
        psg = PS_A.tile([128, 512], F32, name="psg", tag="acc")
        nc.tensor.matmul(psg[:], lhsT=ones128[0:1, :],
                         rhs=vrow[0:1, i * 512:(i + 1) * 512],
                         start=True, stop=True, tile_position=(0, 0))
        nc.scalar.copy(gb[:, i * 512:(i + 1) * 512], psg[:])

    # ---- attention: Q-stationary S tiles, Z via exp-accum, diag numerators.
    # Row/col enumeration within a block: r = c*128 + t_local.
    QT4 = QT.rearrange("p (c hc t) -> p c hc t", c=8, hc=4)
    KT4 = KT.rearrange("p (c hc t) -> p c hc t", c=8, hc=4)
    Bc3 = Bc.rearrange("p (c t) -> p c t", c=8)

    # Per-block work is emitted one block LATE on the tensor engine (numer
    # matmuls and the D chain), so the PE never drains waiting on the
    # trailing exps of the current block (a drained PE gets clock-gated by
    # the HAM and runs at half rate for ~3us after).
    msks = {}
    nmrs = {}

    def numer_mm(a, c):
        nc.tensor.matmul(nmrs[a][:, c:c + 1], lhsT=msks.pop((a, c))[:],
                         rhs=ones_c[:], start=True, stop=True,
                         skip_group_check=True)

    def d_chain(a):
        # D = numer * DS/Z -> transpose to rows 0..7 -> selector-matmul
        # broadcast (all reads at base partition 0; offset bases hang the HW)
        ts = slice(a * 128, (a + 1) * 128)
        nmr = nmrs.pop(a)
        zrt = SCR.tile([128, 8], F32, name="zrt", tag="zrt", bufs=2)
        nc.vector.reciprocal(zrt[:], Zt[:, a * 8:a * 8 + 8])
        nc.vector.tensor_scalar_mul(zrt[:], zrt[:], DS)
        dc = SCR.tile([128, 8], BF16, name="dc", tag="dc", bufs=2)
        nc.vector.tensor_mul(dc[:], nmr[:], zrt[:])
        dt_ps = PS_A.tile([128, 128], BF16, name="dt_ps", tag="acc")
        nc.tensor.transpose(dt_ps[0:8, :], dc[:], identb[:])
        dt_sb = SCR.tile([128, 128], BF16, name="dt_sb", tag="dts", bufs=2)
        nc.vector.tensor_copy(dt_sb[0:8, :], dt_ps[0:8, :])
        bc_ps = PS_S.tile([128, 1024], F32, name="bc_ps", tag="S")
        for c in range(8):
            nc.tensor.matmul(bc_ps[:, c * 128:(c + 1) * 128],
                             lhsT=sel8[0:8, c * 128:(c + 1) * 128],
                             rhs=dt_sb[0:8, :],
                             start=True, stop=True, skip_group_check=True)
        nc.vector.tensor_copy(Bc3[:, :, ts],
                              bc_ps.rearrange("p (c t) -> p c t", c=8))

    for a in range(NBLK):
        ts = slice(a * 128, (a + 1) * 128)
        nmrs[a] = PS_A.tile([128, 8], F32, name="nmr", tag="acc")
        for c in range(8):
            ps = PS_S.tile([128, 1024], F32, name="ps_s", tag="S")
            for half in range(2):
                for pp in range(2):
                    lhsT = QT4[:, c, 2 * pp:2 * pp + 2, ts]
                    rhs = KT4[:, 4 * half:4 * half + 4,
                              2 * pp:2 * pp + 2, ts].transpose([0, 2, 1, 3])
                    nc.tensor.matmul(
                        ps[:, half * 512:(half + 1) * 512],
                        lhsT=lhsT, rhs=rhs,
                        start=(pp == 0), stop=(pp == 1), perf_mode=DR)
            if a > 0:
                numer_mm(a - 1, c)
            junk = SCR.tile([128, 1024], BF16, name="junk", tag="junk", bufs=3)
            nc.scalar.activation(junk[:], ps[:], AF.Exp, scale=ESC,
                                 accum_out=Zt[:, a * 8 + c:a * 8 + c + 1])
            msk = SCR.tile([128, 128], BF16, name="msk", tag="msk", bufs=10)
            nc.vector.tensor_mul(msk[:], junk[:, c * 128:(c + 1) * 128],
                                 identb[:])
            msks[(a, c)] = msk
        if a > 0:
            d_chain(a - 1)
    for c in range(8):
        numer_mm(NBLK - 1, c)
    d_chain(NBLK - 1)

    # ---- V projection fused with diag-softmax scaling -> hhT,
    # then attn out = hhT @ Wo ; + residual ; LN1 core.
    # Chunk pairs share one 2-bank psum so the evac-mult runs at [128,1024]
    # granularity (halves the DVE per-op overhead).
    hhT3 = hhT.rearrange("p (m t) -> p m t", t=512)
    wvsrc = d["wv"].rearrange("(t p) f -> t p f", p=128)
    wvt = {}
    for q in range(4):
        for pp in range(2):
            w = PW.tile([128, 2048], F8, name=f"w_wv{q}{pp}", tag="w")
            nc.sync.dma_start(w[:], wvsrc[pp * 4 + q])
            wvt[(q, pp)] = w
    for mp in range(16):
        ps = PS_S.tile([128, 1024], F32, name="ps_v", tag="S")
        for k in range(2):
            m = 2 * mp + k
            q, mq = m // 8, m % 8
            for pp in range(2):
                lhsT = wvt[(q, pp)].rearrange(
                    "p (j m) -> p j m", j=2)[:, :, mq * 128:(mq + 1) * 128]
                nc.tensor.matmul(ps[:, k * 512:(k + 1) * 512], lhsT=lhsT,
                                 rhs=xTp[pp], start=(pp == 0), stop=(pp == 1),
                                 perf_mode=DR)
        bsl = Bc3[:, mp // 2:mp // 2 + 1, :].broadcast_to((128, 2, 512))
        nc.vector.tensor_mul(
            hhT[:, mp * 1024:(mp + 1) * 1024].rearrange(
                "p (k t) -> p k t", k=2),
            ps.rearrange("p (k t) -> p k t", k=2), bsl)

    def ln_core(v_ap, out_ap):
        nmu = ST.tile([128, 1], F32, name="nmu", tag="nmu")
        nc.vector.reduce_sum(nmu[:], v_ap, axis=AX.X)
        nc.vector.tensor_scalar_mul(nmu[:], nmu[:], -1.0 / H)
        ssq = ST.tile([128, 1], F32, name="ssq", tag="ssq")
        junkf = SCR.tile([128, 512], BF16, name="junkf", tag="junkf", bufs=2)
        nc.scalar.activation(junkf[:], v_ap, AF.Square, bias=nmu[:],
                             accum_out=ssq[:])
        sd = ST.tile([128, 1], F32, name="sd", tag="sd")
        nc.scalar.activation(sd[:], ssq[:], AF.Sqrt, scale=1.0 / H,
                             bias=epsc[:])
        rs = ST.tile([128, 1], F32, name="rs", tag="rs")
        nc.vector.reciprocal(rs[:], sd[:])
        nc.vector.tensor_scalar(out_ap, v_ap, nmu[:], rs[:],
                                op0=ALU.add, op1=ALU.mult)

    def ln(v_ap, gofs, bofs, out_ap):
        ln_core(v_ap, out_ap)
        nc.vector.tensor_mul(out_ap, out_ap, gb[:, gofs * 512:(gofs + 1) * 512])
        nc.vector.tensor_add(out_ap, out_ap, gb[:, bofs * 512:(bofs + 1) * 512])

    wos3 = wos.rearrange("p (i j h) -> p i j h", i=16, j=2)
    for mt in range(4):
        ps_o = PS_A.tile([128, 512], F32, name="ps_o", tag="acc")
        for i in range(16):
            nc.tensor.matmul(
                ps_o[:], lhsT=hhT3[:, 2 * i:2 * i + 2, mt * 128:(mt + 1) * 128],
                rhs=wos3[:, i], start=(i == 0), stop=(i == 15), perf_mode=DR)
        v1 = SCR.tile([128, 512], F32, name="v1", tag="xr")
        nc.vector.scalar_tensor_tensor(
            out=v1[:], in0=ps_o[:], scalar=ODESC,
            in1=xrs[:, mt * 512:(mt + 1) * 512], op0=ALU.mult, op1=ALU.add)
        ln_core(v1[:], xcr[:, mt * 512:(mt + 1) * 512])

    # ---- transpose xcr -> hh1T (fp8) for the FFN (g1/b1 folded into Wf1);
    # hh1r = xcr*g1 + (beta1+bf2) computed off the critical path
    for mt in range(4):
        for jj in range(4):
            tp = PS_A.tile([128, 128], F32, name="tp_h", tag="acc")
            nc.tensor.transpose(
                tp[:], xcr[:, mt * 512 + jj * 128:mt * 512 + jj * 128 + 128],
                identf[:])
            nc.vector.tensor_copy(
                hh1T[:, jj * 512 + mt * 128:jj * 512 + mt * 128 + 128], tp[:])
    for mt in range(4):
        sl = slice(mt * 512, (mt + 1) * 512)
        nc.vector.tensor_mul(hh1r[:, sl], xcr[:, sl], gb[:, 0:512])
        nc.vector.tensor_add(hh1r[:, sl], hh1r[:, sl], gb[:, 512:1024])

    # ---- FFN1 (bf16): a1T = relu(Wf1'^T @ hh1T + bf1')
    for mf in range(16):
        ps = PS_A.tile([128, 512], F32, name="ps_f1", tag="acc")
        for hc in range(4):
            nc.tensor.matmul(
                ps[:], lhsT=wf1s[:, hc * 2048 + mf * 128:hc * 2048 + mf * 128 + 128],
                rhs=hh1T[:, hc * 512:(hc + 1) * 512],
                start=(hc == 0), stop=(hc == 3))
        nc.scalar.activation(a1T[:, mf * 512:(mf + 1) * 512], ps[:], AF.Relu,
                             bias=bf1c[:, mf:mf + 1])

    # ---- FFN2 (bf16) + residual + LN2 -> out
    for mt in range(4):
        ps = PS_A.tile([128, 512], F32, name="ps_f2", tag="acc")
        for fc in range(16):
            nc.tensor.matmul(
                ps[:],
                lhsT=a1T[:, fc * 512 + mt * 128:fc * 512 + mt * 128 + 128],
                rhs=wf2s[:, fc * 512:(fc + 1) * 512],
                start=(fc == 0), stop=(fc == 15))
        s2 = SCR.tile([128, 512], F32, name="s2", tag="xr")
        nc.vector.tensor_add(s2[:], ps[:], hh1r[:, mt * 512:(mt + 1) * 512])
        outt = SCR.tile([128, 512], F32, name="outt", tag="xr")
        ln(s2[:], 2, 3, outt[:])
        nc.sync.dma_start(d["out"][mt * 128:(mt + 1) * 128, :], outt[:])

    for pool in (PS_B, PS_A, PS_S, ST, SCR, PW, P):
        pool.release()


def build(loop_n=None):
    nc = bacc.Bacc("TRN2", target_bir_lowering=False)
    d = {
        "xT": nc.dram_tensor("xT", (TPC, H), F8, kind="ExternalInput").ap(),
        "xr": nc.dram_tensor("xr", (TPC, H), F32, kind="ExternalInput").ap(),
        "wq": nc.dram_tensor("wq", (1024, 2048), F8, kind="ExternalInput").ap(),
        "wk": nc.dram_tensor("wk", (1024, 2048), F8, kind="ExternalInput").ap(),
        "wv": nc.dram_tensor("wv", (1024, 2048), F8, kind="ExternalInput").ap(),
        "wo": nc.dram_tensor("wo", (2048, 1024), F8, kind="ExternalInput").ap(),
        "wf1": nc.dram_tensor("wf1", (H, 4 * H), BF16, kind="ExternalInput").ap(),
        "wf2": nc.dram_tensor("wf2", (4 * H, H), BF16, kind="ExternalInput").ap(),
        "bf1": nc.dram_tensor("bf1", (4 * H,), F32, kind="ExternalInput").ap(),
        "vecs": nc.dram_tensor("vecs", (4, H), BF16, kind="ExternalInput").ap(),
        "ident": nc.dram_tensor("ident", (128, 128), F32,
                                kind="ExternalInput").ap(),
        "sel8": nc.dram_tensor("sel8", (8, 8 * 128), BF16,
                               kind="ExternalInput").ap(),
        "out": nc.dram_tensor("out", (TPC, H), F32, kind="ExternalOutput").ap(),
    }
    with tile.TileContext(nc) as tc:
        if loop_n is None:
            _emit(nc, tc, d)
        else:
            with tc.For_i(0, loop_n, 1):
                _emit(nc, tc, d)
    nc.finalize()
    return nc


def _pack_w(W):
    # tile-major fp8 packing for DoubleRow projections:
    # tile (pair, q): [p, j*1024+m] = W[(2*pair+j)*128+p, q*1024+m]
    W5 = np.asarray(W, np.float32).reshape(2, 2, 128, 4, 1024)
    return np.ascontiguousarray(
        W5.transpose(0, 3, 2, 1, 4).reshape(8 * 128, 2048)).astype(_F8)


def _pack_wo(W):
    # tile i: [p, j*512+h] = Wo[(2*i+j)*128+p, h]
    W4 = np.asarray(W, np.float32).reshape(16, 2, 128, 512)
    return np.ascontiguousarray(
        W4.transpose(0, 2, 1, 3).reshape(16 * 128, 1024)).astype(_F8)


def make_in_maps(inputs):
    xf = np.ascontiguousarray(
        np.asarray(inputs["x"], np.float32).reshape(TOK, H))
    g1 = np.asarray(inputs["g1"], np.float32)
    wf1 = np.asarray(inputs["Wf1"], np.float32)
    shared = {
        "wq": _pack_w(WS * np.asarray(inputs["Wq"], np.float32)),
        "wk": _pack_w(WS * np.asarray(inputs["Wk"], np.float32)),
        "wv": _pack_w(WS * np.asarray(inputs["Wv"], np.float32)),
        "wo": _pack_wo(WS * np.asarray(inputs["Wo"], np.float32)),
        "wf1": (g1[:, None] * wf1).astype(_BF),
        "wf2": np.asarray(inputs["Wf2"], np.float32).astype(_BF),
        "bf1": (np.asarray(inputs["bf1"], np.float32)
                + np.asarray(inputs["beta1"], np.float32) @ wf1),
        "vecs": np.ascontiguousarray(np.stack([
            g1,
            np.asarray(inputs["beta1"], np.float32)
            + np.asarray(inputs["bf2"], np.float32),
            np.asarray(inputs["g2"], np.float32),
            np.asarray(inputs["beta2"], np.float32)]).astype(_BF)),
        "ident": np.eye(128, dtype=np.float32),
        "sel8": np.ascontiguousarray(
            np.kron(np.eye(8, dtype=np.float32), np.ones((1, 128), np.float32))
            .astype(_BF)),
    }
    in_maps = []
    for c in range(NCORES):
        xs = xf[c * TPC:(c + 1) * TPC]
        m = dict(shared)
        m["xT"] = np.ascontiguousarray(xs.T).astype(_F8)
        m["xr"] = np.ascontiguousarray(xs)
        in_maps.append(m)
    return in_maps


_nc_cache = None


def _get_nc():
    global _nc_cache
    if _nc_cache is None:
        _nc_cache = build()
    return _nc_cache


def kernel(**inputs):
    nc = _get_nc()
    in_maps = make_in_maps(inputs)
    res = bass_utils.run_bass_kernel_spmd(nc, in_maps,
                                          core_ids=list(range(NCORES)))
    out = np.concatenate([r["out"] for r in res.results], axis=0)
    return out.reshape(B, T, H)


if __name__ == "__main__":
    nc = build()
    n_inst = sum(len(bb.instructions) for bb in nc.main_func.blocks)
    print("built OK; instructions:", n_inst)


# revision 29
# speedup vs baseline: 1.2072x; 1.0391x over previous
"""Trainium2 Bass kernel for the MultiHeadAttention-variant transformer block.

Math notes (derived from the module semantics):
  - The einsum 'batt,bath->bath' uses only the DIAGONAL of the softmax'd
    attention matrix, so per flat row i the attention output is
    softmax_diag_i * V[i].
  - The raw reshape (B,T,N*H)->(B,N,T,H) makes attention "blocks" couple only
    groups of 128 consecutive tokens (T/N = 1024/8 = 128); a block's 1024
    logical rows are the (chunk c, token t) pairs of those 128 tokens.
  => The whole layer is data-parallel over 128-token groups. We shard the
     4096 flattened tokens as 512 consecutive tokens per core (8 cores), with
     zero cross-core communication.

v2 design (vs the v1 baseline):
  - S is computed Q-STATIONARY with rows enumerated r = c*128 + t, so each
    row-tile of the 1024x1024 block-attention matrix is one out-psum
    [128, 1024] whose FREE axis spans the full softmax denominator.  The
    row sums Z then come for free from the Exp activation's accum_out, and
    the numerators are the diagonals of the c-th 128-col group (identity
    mask * ones-column matmul).  This removes the v1 Z-row matmul chains,
    the [1,512] reciprocals (28us of DVE!), and most broadcast plumbing.
  - All five weight matmuls (QKV, Wo, FFN1, FFN2) run fp8e4m3 DoubleRow.
    Weights are pre-scaled by 16 on the host (values ~0.02*N(0,1) would
    otherwise sit at the bottom of the fp8 range); the scales are folded
    into the exp() activation scale and the two residual descales.
  - LDWEIGHTS is fully hidden behind 512-col DR matmuls (216ns cadence),
    so the tensor-engine floor is the streamed column count.
"""

import sys

sys.path.insert(0, "/opt/trn_rl_repo")

import numpy as np
import ml_dtypes

import concourse.bass as bass
import concourse.mybir as mybir
import concourse.tile as tile
from concourse import bacc, bass_utils

F32 = mybir.dt.float32
BF16 = mybir.dt.bfloat16
F8 = mybir.dt.float8e4
AF = mybir.ActivationFunctionType
ALU = mybir.AluOpType
AX = mybir.AxisListType
DR = mybir.MatmulPerfMode.DoubleRow

H = 512
NH = 8
B = 4
T = 1024
TOK = B * T
NCORES = 8
TPC = TOK // NCORES  # 512 tokens per core
NBLK = TPC // 128  # 4 attention blocks per core
SCALE = float(1.0 / np.sqrt(H))
LN_EPS = 1e-5

_BF = ml_dtypes.bfloat16
_F8 = ml_dtypes.float8_e4m3

WS = 16.0  # host pre-scale on every weight matrix (fp8 range)
DS = 64.0  # extra scale on the softmax-diag D (fp8 range of hhT)
ESC = SCALE / (WS * WS)  # exp() input scale: undo Wq*16 * Wk*16
ODESC = 1.0 / (WS * WS * DS)  # Wo-path descale: V*16, Wo*16, D*64
FDESC = 1.0 / WS  # FFN descales


def _emit(nc, tc, d):
    """Emit the per-core program. d: dict of DRAM APs."""
    P = tc.alloc_tile_pool(name="persist", bufs=1)
    PW = tc.alloc_tile_pool(name="wpool", bufs=11)
    SCR = tc.alloc_tile_pool(name="scr", bufs=4)
    ST = tc.alloc_tile_pool(name="stats", bufs=4)
    PS_S = tc.alloc_tile_pool(name="ps_s", bufs=2, space="PSUM")
    PS_A = tc.alloc_tile_pool(name="ps_a", bufs=4, space="PSUM")

    # ---- persistent tiles
    xT = P.tile([128, 4 * 512], F8, name="xT")  # x^T, 4 h-chunks
    identf = P.tile([128, 128], F32, name="identf")
    identb = P.tile([128, 128], BF16, name="identb")
    ones_c = P.tile([128, 1], BF16, name="ones_c")  # rhs for diag-col matmul
    ones128 = P.tile([128, 128], BF16, name="ones128")  # bcast lhsT rows
    vrow = P.tile([1, 4 * 512], BF16, name="vrow")  # g1,vsum,g2,b2 rows
    bf1c = P.tile([128, 16], F32, name="bf1c")  # bf1 column-major
    gb = P.tile([128, 4 * 512], F32, name="gb")  # bcast g1,vsum,g2,b2
    Bc = P.tile([128, 8 * 512], BF16, name="Bc")  # D*DS bcast, [p, c*512+t]
    Zt = P.tile([128, 32], F32, name="Zt")  # softmax denoms, col a*8+c
    epsc = P.tile([128, 1], F32, name="epsc")
    xrs = P.tile([128, 4 * 512], F32, name="xrs")  # residual x, [p, mt*512+h]
    sel8 = P.tile([128, 8 * 128], BF16, name="sel8")  # sel8[k,c*128+p]=(k==c)
    QT = P.tile([128, 32 * 512], F8, name="QT")
    KT = P.tile([128, 32 * 512], F8, name="KT")
    hhT = P.tile([128, 32 * 512], F8, name="hhT")
    xcr = P.tile([128, 4 * 512], F32, name="xcr")  # LN1 core out (no g/b)
    hh1T = P.tile([128, 4 * 512], BF16, name="hh1T")  # xcr^T for FFN1
    hh1r = P.tile([128, 4 * 512], F32, name="hh1r")  # true hh1 residual
    a1T = P.tile([128, 16 * 512], BF16, name="a1T")  # relu acts, [p, fc*512+t]
    wos = P.tile([128, 16 * 1024], F8, name="wos")  # Wo DR-packed
    wf1s = P.tile([128, 4 * 2048], BF16, name="wf1s")  # [p, hc*2048+f]
    wf2s = P.tile([128, 16 * 512], BF16, name="wf2s")  # [p, fc*512+h]

    # ---- input DMAs.  sync ring: critical path (x, ident, wq/wk/wv stream).
    # scalar ring: everything needed later (vecs, bf1, xr, wo, wf1, wf2).
    nc.sync.dma_start(
        xT.rearrange("p (hc t) -> p hc t", hc=4),
        d["xT"].rearrange("(hc p) t -> p hc t", p=128))
    nc.sync.dma_start(identf[:], d["ident"][:])
    nc.scalar.dma_start(vrow[0:1, :], d["vecs"].rearrange("v h -> (v h)")[None, :])
    nc.scalar.dma_start(bf1c[:], d["bf1"].rearrange("(m p) -> p m", p=128))

    nc.vector.memset(ones_c[:], 1.0)
    nc.vector.memset(ones128[:], 1.0)
    nc.vector.memset(epsc[:], LN_EPS)
    nc.vector.tensor_copy(identb[:], identf[:])

    def late_dmas():
        # big late-deadline weights go on the (otherwise idle) gpsimd SWDGE
        # ring, gated behind the QK weight stream (emitted mid-phase-A) so
        # their 7MB doesn't steal fabric bandwidth from the critical path
        nc.gpsimd.dma_start(sel8[0:8, :], d["sel8"][:])
        nc.gpsimd.dma_start(
            xrs.rearrange("p (mt h) -> p mt h", mt=4),
            d["xr"].rearrange("(mt p) h -> p mt h", p=128))
        nc.gpsimd.dma_start(
            wos.rearrange("p (i f) -> p i f", i=16),
            d["wo"].rearrange("(i p) f -> p i f", p=128))
        nc.gpsimd.dma_start(
            wf1s.rearrange("p (hc f) -> p hc f", hc=4),
            d["wf1"].rearrange("(hc p) f -> p hc f", p=128))
        nc.gpsimd.dma_start(
            wf2s.rearrange("p (fc h) -> p fc h", fc=16),
            d["wf2"].rearrange("(fc p) h -> p fc h", p=128))

    # ---- projections (fp8 DoubleRow): dst^T = W^T @ x^T.  Weight DRAM is
    # pre-packed tile-major: tile (pair, q) rows,
    # [p, j*1024+m] = W[(2*pair+j)*128+p, q*1024+m].
    xTp = [xT[:, pp * 1024:(pp + 1) * 1024].rearrange("p (j t) -> p j t", j=2)
           for pp in range(2)]

    def proj(wname, evac):
        wsrc = d[wname].rearrange("(t p) f -> t p f", p=128)
        wt = {}
        for q in range(4):
            for pp in range(2):
                w = PW.tile([128, 2048], F8, name=f"w_{wname}{q}{pp}", tag="w")
                nc.sync.dma_start(w[:], wsrc[pp * 4 + q])
                wt[(q, pp)] = w
        for m in range(32):
            q, mq = m // 8, m % 8
            ps = PS_A.tile([128, 512], F32, name="ps_proj", tag="acc")
            for pp in range(2):
                lhsT = wt[(q, pp)].rearrange(
                    "p (j m) -> p j m", j=2)[:, :, mq * 128:(mq + 1) * 128]
                nc.tensor.matmul(ps[:], lhsT=lhsT, rhs=xTp[pp],
                                 start=(pp == 0), stop=(pp == 1),
                                 perf_mode=DR)
            evac(m, ps)

    def evac_alt(dst):
        def f(m, ps):
            sl = dst[:, m * 512:(m + 1) * 512]
            if m % 2 == 0:
                nc.vector.tensor_copy(sl, ps[:])
            else:
                nc.scalar.copy(sl, ps[:])
        return f

    proj("wq", evac_alt(QT))
    # data-dependency gate: gpsimd waits for the end of the Q projection
    # before issuing its bulk prefetches
    gdum = P.tile([128, 1], F8, name="gdum")
    nc.gpsimd.tensor_copy(gdum[:], QT[:, 16383:16384])
    late_dmas()
    proj("wk", evac_alt(KT))

    # ---- gamma/beta broadcast rows -> gb
    for i in range(4):
        psg = PS_A.tile([128, 512], F32, name="psg", tag="acc")
        nc.tensor.matmul(psg[:], lhsT=ones128[0:1, :],
                         rhs=vrow[0:1, i * 512:(i + 1) * 512],
                         start=True, stop=True, tile_position=(0, 0))
        nc.scalar.copy(gb[:, i * 512:(i + 1) * 512], psg[:])

    # ---- attention: Q-stationary S tiles, Z via exp-accum, diag numerators.
    # Row/col enumeration within a block: r = c*128 + t_local.
    QT4 = QT.rearrange("p (c hc t) -> p c hc t", c=8, hc=4)
    KT4 = KT.rearrange("p (c hc t) -> p c hc t", c=8, hc=4)
    Bc3 = Bc.rearrange("p (c t) -> p c t", c=8)

    # Per-block work is emitted one block LATE on the tensor engine (numer
    # matmuls and the D chain), so the PE never drains waiting on the
    # trailing exps of the current block (a drained PE gets clock-gated by
    # the HAM and runs at half rate for ~3us after).
    msks = {}
    nmrs = {}

    def numer_mm(a, c):
        nc.tensor.matmul(nmrs[a][:, c:c + 1], lhsT=msks.pop((a, c))[:],
                         rhs=ones_c[:], start=True, stop=True,
                         skip_group_check=True)

    def d_chain(a):
        # D = numer * DS/Z -> transpose to rows 0..7 -> selector-matmul
        # broadcast (all reads at base partition 0; offset bases hang the HW)
        ts = slice(a * 128, (a + 1) * 128)
        nmr = nmrs.pop(a)
        zrt = SCR.tile([128, 8], F32, name="zrt", tag="zrt", bufs=2)
        nc.vector.reciprocal(zrt[:], Zt[:, a * 8:a * 8 + 8])
        nc.vector.tensor_scalar_mul(zrt[:], zrt[:], DS)
        dc = SCR.tile([128, 8], BF16, name="dc", tag="dc", bufs=2)
        nc.vector.tensor_mul(dc[:], nmr[:], zrt[:])
        dt_ps = PS_A.tile([128, 128], BF16, name="dt_ps", tag="acc")
        nc.tensor.transpose(dt_ps[0:8, :], dc[:], identb[:])
        dt_sb = SCR.tile([128, 128], BF16, name="dt_sb", tag="dts", bufs=2)
        nc.vector.tensor_copy(dt_sb[0:8, :], dt_ps[0:8, :])
        bc_ps = PS_S.tile([128, 1024], F32, name="bc_ps", tag="S")
        for c in range(8):
            nc.tensor.matmul(bc_ps[:, c * 128:(c + 1) * 128],
                             lhsT=sel8[0:8, c * 128:(c + 1) * 128],
                             rhs=dt_sb[0:8, :],
                             start=True, stop=True, skip_group_check=True)
        nc.vector.tensor_copy(Bc3[:, :, ts],
                              bc_ps.rearrange("p (c t) -> p c t", c=8))

    for a in range(NBLK):
        ts = slice(a * 128, (a + 1) * 128)
        nmrs[a] = PS_A.tile([128, 8], F32, name="nmr", tag="acc")
        for c in range(8):
            ps = PS_S.tile([128, 1024], F32, name="ps_s", tag="S")
            for half in range(2):
                for pp in range(2):
                    lhsT = QT4[:, c, 2 * pp:2 * pp + 2, ts]
                    rhs = KT4[:, 4 * half:4 * half + 4,
                              2 * pp:2 * pp + 2, ts].transpose([0, 2, 1, 3])
                    nc.tensor.matmul(
                        ps[:, half * 512:(half + 1) * 512],
                        lhsT=lhsT, rhs=rhs,
                        start=(pp == 0), stop=(pp == 1), perf_mode=DR)
            if a > 0:
                numer_mm(a - 1, c)
            junk = SCR.tile([128, 1024], BF16, name="junk", tag="junk", bufs=3)
            nc.scalar.activation(junk[:], ps[:], AF.Exp, scale=ESC,
                                 accum_out=Zt[:, a * 8 + c:a * 8 + c + 1])
            msk = SCR.tile([128, 128], BF16, name="msk", tag="msk", bufs=10)
            nc.vector.tensor_mul(msk[:], junk[:, c * 128:(c + 1) * 128],
                                 identb[:])
            msks[(a, c)] = msk
        if a > 0:
            d_chain(a - 1)
    for c in range(8):
        numer_mm(NBLK - 1, c)
    d_chain(NBLK - 1)

    # ---- V projection fused with diag-softmax scaling -> hhT,
    # then attn out = hhT @ Wo ; + residual ; LN1 core.
    # Chunk pairs share one 2-bank psum so the evac-mult runs at [128,1024]
    # granularity (halves the DVE per-op overhead).
    hhT3 = hhT.rearrange("p (m t) -> p m t", t=512)
    wvsrc = d["wv"].rearrange("(t p) f -> t p f", p=128)
    wvt = {}
    for q in range(4):
        for pp in range(2):
            w = PW.tile([128, 2048], F8, name=f"w_wv{q}{pp}", tag="w")
            nc.sync.dma_start(w[:], wvsrc[pp * 4 + q])
            wvt[(q, pp)] = w
    for mp in range(16):
        ps = PS_S.tile([128, 1024], F32, name="ps_v", tag="S")
        for k in range(2):
            m = 2 * mp + k
            q, mq = m // 8, m % 8
            for pp in range(2):
                lhsT = wvt[(q, pp)].rearrange(
                    "p (j m) -> p j m", j=2)[:, :, mq * 128:(mq + 1) * 128]
                nc.tensor.matmul(ps[:, k * 512:(k + 1) * 512], lhsT=lhsT,
                                 rhs=xTp[pp], start=(pp == 0), stop=(pp == 1),
                                 perf_mode=DR)
        bsl = Bc3[:, mp // 2:mp // 2 + 1, :].broadcast_to((128, 2, 512))
        nc.vector.tensor_mul(
            hhT[:, mp * 1024:(mp + 1) * 1024].rearrange(
                "p (k t) -> p k t", k=2),
            ps.rearrange("p (k t) -> p k t", k=2), bsl)

    def ln_core(v_ap, out_ap):
        nmu = ST.tile([128, 1], F32, name="nmu", tag="nmu")
        nc.vector.reduce_sum(nmu[:], v_ap, axis=AX.X)
        nc.vector.tensor_scalar_mul(nmu[:], nmu[:], -1.0 / H)
        ssq = ST.tile([128, 1], F32, name="ssq", tag="ssq")
        junkf = SCR.tile([128, 512], BF16, name="junkf", tag="junkf", bufs=2)
        nc.scalar.activation(junkf[:], v_ap, AF.Square, bias=nmu[:],
                             accum_out=ssq[:])
        sd = ST.tile([128, 1], F32, name="sd", tag="sd")
        nc.scalar.activation(sd[:], ssq[:], AF.Sqrt, scale=1.0 / H,
                             bias=epsc[:])
        rs = ST.tile([128, 1], F32, name="rs", tag="rs")
        nc.vector.reciprocal(rs[:], sd[:])
        nc.vector.tensor_scalar(out_ap, v_ap, nmu[:], rs[:],
                                op0=ALU.add, op1=ALU.mult)

    def ln(v_ap, gofs, bofs, out_ap):
        ln_core(v_ap, out_ap)
        nc.vector.tensor_mul(out_ap, out_ap, gb[:, gofs * 512:(gofs + 1) * 512])
        nc.vector.tensor_add(out_ap, out_ap, gb[:, bofs * 512:(bofs + 1) * 512])

    # Wo chains interleaved with the xcr->hh1T transposes (fp32, for FFN1)
    # so the PE never drains waiting on a trailing LN1 chain.
    wos3 = wos.rearrange("p (i j h) -> p i j h", i=16, j=2)

    def wo_chain(mt):
        ps_o = PS_A.tile([128, 512], F32, name="ps_o", tag="acc")
        for i in range(16):
            nc.tensor.matmul(
                ps_o[:], lhsT=hhT3[:, 2 * i:2 * i + 2, mt * 128:(mt + 1) * 128],
                rhs=wos3[:, i], start=(i == 0), stop=(i == 15), perf_mode=DR)
        v1 = SCR.tile([128, 512], F32, name="v1", tag="xr")
        nc.vector.scalar_tensor_tensor(
            out=v1[:], in0=ps_o[:], scalar=ODESC,
            in1=xrs[:, mt * 512:(mt + 1) * 512], op0=ALU.mult, op1=ALU.add)
        ln_core(v1[:], xcr[:, mt * 512:(mt + 1) * 512])

    def xcr_transpose(mt):
        for jj in range(4):
            tp = PS_A.tile([128, 128], F32, name="tp_h", tag="acc")
            nc.tensor.transpose(
                tp[:], xcr[:, mt * 512 + jj * 128:mt * 512 + jj * 128 + 128],
                identf[:])
            nc.vector.tensor_copy(
                hh1T[:, jj * 512 + mt * 128:jj * 512 + mt * 128 + 128], tp[:])
        sl = slice(mt * 512, (mt + 1) * 512)
        nc.vector.tensor_mul(hh1r[:, sl], xcr[:, sl], gb[:, 0:512])
        nc.vector.tensor_add(hh1r[:, sl], hh1r[:, sl], gb[:, 512:1024])

    wo_chain(0)
    wo_chain(1)
    xcr_transpose(0)
    wo_chain(2)
    xcr_transpose(1)
    wo_chain(3)
    xcr_transpose(2)
    xcr_transpose(3)

    # ---- FFN1 (bf16): a1T = relu(Wf1'^T @ hh1T + bf1')
    for mf in range(16):
        ps = PS_A.tile([128, 512], F32, name="ps_f1", tag="acc")
        for hc in range(4):
            nc.tensor.matmul(
                ps[:], lhsT=wf1s[:, hc * 2048 + mf * 128:hc * 2048 + mf * 128 + 128],
                rhs=hh1T[:, hc * 512:(hc + 1) * 512],
                start=(hc == 0), stop=(hc == 3))
        nc.scalar.activation(a1T[:, mf * 512:(mf + 1) * 512], ps[:], AF.Relu,
                             bias=bf1c[:, mf:mf + 1])

    # ---- FFN2 (bf16) + residual + LN2 -> out
    for mt in range(4):
        ps = PS_A.tile([128, 512], F32, name="ps_f2", tag="acc")
        for fc in range(16):
            nc.tensor.matmul(
                ps[:],
                lhsT=a1T[:, fc * 512 + mt * 128:fc * 512 + mt * 128 + 128],
                rhs=wf2s[:, fc * 512:(fc + 1) * 512],
                start=(fc == 0), stop=(fc == 15))
        s2 = SCR.tile([128, 512], F32, name="s2", tag="xr")
        nc.vector.tensor_add(s2[:], ps[:], hh1r[:, mt * 512:(mt + 1) * 512])
        outt = SCR.tile([128, 512], F32, name="outt", tag="xr")
        ln(s2[:], 2, 3, outt[:])
        nc.sync.dma_start(d["out"][mt * 128:(mt + 1) * 128, :], outt[:])

    for pool in (PS_B, PS_A, PS_S, ST, SCR, PW, P):
        pool.release()


def build(loop_n=None):
    nc = bacc.Bacc("TRN2", target_bir_lowering=False)
    d = {
        "xT": nc.dram_tensor("xT", (TPC, H), F8, kind="ExternalInput").ap(),
        "xr": nc.dram_tensor("xr", (TPC, H), F32, kind="ExternalInput").ap(),
        "wq": nc.dram_tensor("wq", (1024, 2048), F8, kind="ExternalInput").ap(),
        "wk": nc.dram_tensor("wk", (1024, 2048), F8, kind="ExternalInput").ap(),
        "wv": nc.dram_tensor("wv", (1024, 2048), F8, kind="ExternalInput").ap(),
        "wo": nc.dram_tensor("wo", (2048, 1024), F8, kind="ExternalInput").ap(),
        "wf1": nc.dram_tensor("wf1", (H, 4 * H), BF16, kind="ExternalInput").ap(),
        "wf2": nc.dram_tensor("wf2", (4 * H, H), BF16, kind="ExternalInput").ap(),
        "bf1": nc.dram_tensor("bf1", (4 * H,), F32, kind="ExternalInput").ap(),
        "vecs": nc.dram_tensor("vecs", (4, H), BF16, kind="ExternalInput").ap(),
        "ident": nc.dram_tensor("ident", (128, 128), F32,
                                kind="ExternalInput").ap(),
        "sel8": nc.dram_tensor("sel8", (8, 8 * 128), BF16,
                               kind="ExternalInput").ap(),
        "out": nc.dram_tensor("out", (TPC, H), F32, kind="ExternalOutput").ap(),
    }
    with tile.TileContext(nc) as tc:
        if loop_n is None:
            _emit(nc, tc, d)
        else:
            with tc.For_i(0, loop_n, 1):
                _emit(nc, tc, d)
    nc.finalize()
    return nc


def _pack_w(W):
    # tile-major fp8 packing for DoubleRow projections:
    # tile (pair, q): [p, j*1024+m] = W[(2*pair+j)*128+p, q*1024+m]
    W5 = np.asarray(W, np.float32).reshape(2, 2, 128, 4, 1024)
    return np.ascontiguousarray(
        W5.transpose(0, 3, 2, 1, 4).reshape(8 * 128, 2048)).astype(_F8)


def _pack_wo(W):
    # tile i: [p, j*512+h] = Wo[(2*i+j)*128+p, h]
    W4 = np.asarray(W, np.float32).reshape(16, 2, 128, 512)
    return np.ascontiguousarray(
        W4.transpose(0, 2, 1, 3).reshape(16 * 128, 1024)).astype(_F8)


def make_in_maps(inputs):
    xf = np.ascontiguousarray(
        np.asarray(inputs["x"], np.float32).reshape(TOK, H))
    g1 = np.asarray(inputs["g1"], np.float32)
    wf1 = np.asarray(inputs["Wf1"], np.float32)
    shared = {
        "wq": _pack_w(WS * np.asarray(inputs["Wq"], np.float32)),
        "wk": _pack_w(WS * np.asarray(inputs["Wk"], np.float32)),
        "wv": _pack_w(WS * np.asarray(inputs["Wv"], np.float32)),
        "wo": _pack_wo(WS * np.asarray(inputs["Wo"], np.float32)),
        "wf1": (g1[:, None] * wf1).astype(_BF),
        "wf2": np.asarray(inputs["Wf2"], np.float32).astype(_BF),
        "bf1": (np.asarray(inputs["bf1"], np.float32)
                + np.asarray(inputs["beta1"], np.float32) @ wf1),
        "vecs": np.ascontiguousarray(np.stack([
            g1,
            np.asarray(inputs["beta1"], np.float32)
            + np.asarray(inputs["bf2"], np.float32),
            np.asarray(inputs["g2"], np.float32),
            np.asarray(inputs["beta2"], np.float32)]).astype(_BF)),
        "ident": np.eye(128, dtype=np.float32),
        "sel8": np.ascontiguousarray(
            np.kron(np.eye(8, dtype=np.float32), np.ones((1, 128), np.float32))
            .astype(_BF)),
    }
    in_maps = []
    for c in range(NCORES):
        xs = xf[c * TPC:(c + 1) * TPC]
        m = dict(shared)
        m["xT"] = np.ascontiguousarray(xs.T).astype(_F8)
        m["xr"] = np.ascontiguousarray(xs)
        in_maps.append(m)
    return in_maps


_nc_cache = None


def _get_nc():
    global _nc_cache
    if _nc_cache is None:
        _nc_cache = build()
    return _nc_cache


def kernel(**inputs):
    nc = _get_nc()
    in_maps = make_in_maps(inputs)
    res = bass_utils.run_bass_kernel_spmd(nc, in_maps,
                                          core_ids=list(range(NCORES)))
    out = np.concatenate([r["out"] for r in res.results], axis=0)
    return out.reshape(B, T, H)


if __name__ == "__main__":
    nc = build()
    n_inst = sum(len(bb.instructions) for bb in nc.main_func.blocks)
    print("built OK; instructions:", n_inst)


# revision 32
# speedup vs baseline: 1.2113x; 1.0034x over previous
"""Trainium2 Bass kernel for the MultiHeadAttention-variant transformer block.

Math notes (derived from the module semantics):
  - The einsum 'batt,bath->bath' uses only the DIAGONAL of the softmax'd
    attention matrix, so per flat row i the attention output is
    softmax_diag_i * V[i].
  - The raw reshape (B,T,N*H)->(B,N,T,H) makes attention "blocks" couple only
    groups of 128 consecutive tokens (T/N = 1024/8 = 128); a block's 1024
    logical rows are the (chunk c, token t) pairs of those 128 tokens.
  => The whole layer is data-parallel over 128-token groups. We shard the
     4096 flattened tokens as 512 consecutive tokens per core (8 cores), with
     zero cross-core communication.

v2 design (vs the v1 baseline):
  - S is computed Q-STATIONARY with rows enumerated r = c*128 + t, so each
    row-tile of the 1024x1024 block-attention matrix is one out-psum
    [128, 1024] whose FREE axis spans the full softmax denominator.  The
    row sums Z then come for free from the Exp activation's accum_out, and
    the numerators are the diagonals of the c-th 128-col group (identity
    mask * ones-column matmul).  This removes the v1 Z-row matmul chains,
    the [1,512] reciprocals (28us of DVE!), and most broadcast plumbing.
  - All five weight matmuls (QKV, Wo, FFN1, FFN2) run fp8e4m3 DoubleRow.
    Weights are pre-scaled by 16 on the host (values ~0.02*N(0,1) would
    otherwise sit at the bottom of the fp8 range); the scales are folded
    into the exp() activation scale and the two residual descales.
  - LDWEIGHTS is fully hidden behind 512-col DR matmuls (216ns cadence),
    so the tensor-engine floor is the streamed column count.
"""

import sys

sys.path.insert(0, "/opt/trn_rl_repo")

import numpy as np
import ml_dtypes

import concourse.bass as bass
import concourse.mybir as mybir
import concourse.tile as tile
from concourse import bacc, bass_utils

F32 = mybir.dt.float32
BF16 = mybir.dt.bfloat16
F8 = mybir.dt.float8e4
AF = mybir.ActivationFunctionType
ALU = mybir.AluOpType
AX = mybir.AxisListType
DR = mybir.MatmulPerfMode.DoubleRow

H = 512
NH = 8
B = 4
T = 1024
TOK = B * T
NCORES = 8
TPC = TOK // NCORES  # 512 tokens per core
NBLK = TPC // 128  # 4 attention blocks per core
SCALE = float(1.0 / np.sqrt(H))
LN_EPS = 1e-5

_BF = ml_dtypes.bfloat16
_F8 = ml_dtypes.float8_e4m3

WS = 16.0  # host pre-scale on every weight matrix (fp8 range)
DS = 64.0  # extra scale on the softmax-diag D (fp8 range of hhT)
ESC = SCALE / (WS * WS)  # exp() input scale: undo Wq*16 * Wk*16
ODESC = 1.0 / (WS * WS * DS)  # Wo-path descale: V*16, Wo*16, D*64
FDESC = 1.0 / WS  # FFN descales


def _emit(nc, tc, d):
    """Emit the per-core program. d: dict of DRAM APs."""
    P = tc.alloc_tile_pool(name="persist", bufs=1)
    PW = tc.alloc_tile_pool(name="wpool", bufs=11)
    SCR = tc.alloc_tile_pool(name="scr", bufs=4)
    ST = tc.alloc_tile_pool(name="stats", bufs=4)
    PS_S = tc.alloc_tile_pool(name="ps_s", bufs=2, space="PSUM")
    PS_A = tc.alloc_tile_pool(name="ps_a", bufs=4, space="PSUM")

    # ---- persistent tiles
    xT = P.tile([128, 4 * 512], F8, name="xT")  # x^T, 4 h-chunks
    identf = P.tile([128, 128], F32, name="identf")
    identb = P.tile([128, 128], BF16, name="identb")
    ones_c = P.tile([128, 1], BF16, name="ones_c")  # rhs for diag-col matmul
    ones128 = P.tile([128, 128], BF16, name="ones128")  # bcast lhsT rows
    vrow = P.tile([1, 4 * 512], BF16, name="vrow")  # g1,vsum,g2,b2 rows
    bf1c = P.tile([128, 16], F32, name="bf1c")  # bf1 column-major
    gb = P.tile([128, 4 * 512], F32, name="gb")  # bcast g1,vsum,g2,b2
    Bc = P.tile([128, 8 * 512], BF16, name="Bc")  # D*DS bcast, [p, c*512+t]
    Zt = P.tile([128, 32], F32, name="Zt")  # softmax denoms, col a*8+c
    epsc = P.tile([128, 1], F32, name="epsc")
    xrs = P.tile([128, 4 * 512], F32, name="xrs")  # residual x, [p, mt*512+h]
    sel8 = P.tile([128, 8 * 128], BF16, name="sel8")  # sel8[k,c*128+p]=(k==c)
    QT = P.tile([128, 32 * 512], F8, name="QT")
    KT = P.tile([128, 32 * 512], F8, name="KT")
    hhT = P.tile([128, 32 * 512], F8, name="hhT")
    xcr = P.tile([128, 4 * 512], F32, name="xcr")  # LN1 core out (no g/b)
    hh1T = P.tile([128, 4 * 512], BF16, name="hh1T")  # xcr^T for FFN1
    hh1r = P.tile([128, 4 * 512], F32, name="hh1r")  # true hh1 residual
    a1T = P.tile([128, 16 * 512], BF16, name="a1T")  # relu acts, [p, fc*512+t]
    wos = P.tile([128, 16 * 1024], F8, name="wos")  # Wo DR-packed
    wf1s = P.tile([128, 4 * 2048], BF16, name="wf1s")  # [p, hc*2048+f]
    wf2s = P.tile([128, 16 * 512], BF16, name="wf2s")  # [p, fc*512+h]

    # ---- input DMAs.  sync ring: critical path (x, ident, wq/wk/wv stream).
    # scalar ring: everything needed later (vecs, bf1, xr, wo, wf1, wf2).
    nc.sync.dma_start(
        xT.rearrange("p (hc t) -> p hc t", hc=4),
        d["xT"].rearrange("(hc p) t -> p hc t", p=128))
    nc.sync.dma_start(identf[:], d["ident"][:])
    nc.scalar.dma_start(vrow[0:1, :], d["vecs"].rearrange("v h -> (v h)")[None, :])
    nc.scalar.dma_start(bf1c[:], d["bf1"].rearrange("(m p) -> p m", p=128))

    nc.vector.memset(ones_c[:], 1.0)
    nc.vector.memset(ones128[:], 1.0)
    nc.vector.memset(epsc[:], LN_EPS)
    nc.vector.tensor_copy(identb[:], identf[:])

    def late_dmas():
        # big late-deadline weights go on the (otherwise idle) gpsimd SWDGE
        # ring, gated behind the QK weight stream (emitted mid-phase-A) so
        # their 7MB doesn't steal fabric bandwidth from the critical path
        nc.gpsimd.dma_start(sel8[0:8, :], d["sel8"][:])
        nc.gpsimd.dma_start(
            xrs.rearrange("p (mt h) -> p mt h", mt=4),
            d["xr"].rearrange("(mt p) h -> p mt h", p=128))
        nc.gpsimd.dma_start(
            wos.rearrange("p (i f) -> p i f", i=16),
            d["wo"].rearrange("(i p) f -> p i f", p=128))
        nc.gpsimd.dma_start(
            wf1s.rearrange("p (hc f) -> p hc f", hc=4),
            d["wf1"].rearrange("(hc p) f -> p hc f", p=128))
        nc.gpsimd.dma_start(
            wf2s.rearrange("p (fc h) -> p fc h", fc=16),
            d["wf2"].rearrange("(fc p) h -> p fc h", p=128))

    # ---- projections (fp8 DoubleRow): dst^T = W^T @ x^T.  Weight DRAM is
    # pre-packed tile-major: tile (pair, q) rows,
    # [p, j*1024+m] = W[(2*pair+j)*128+p, q*1024+m].
    xTp = [xT[:, pp * 1024:(pp + 1) * 1024].rearrange("p (j t) -> p j t", j=2)
           for pp in range(2)]

    def proj(wname, evac):
        wsrc = d[wname].rearrange("(t p) f -> t p f", p=128)
        wt = {}
        for q in range(4):
            for pp in range(2):
                w = PW.tile([128, 2048], F8, name=f"w_{wname}{q}{pp}", tag="w")
                nc.sync.dma_start(w[:], wsrc[pp * 4 + q])
                wt[(q, pp)] = w
        for m in range(32):
            q, mq = m // 8, m % 8
            ps = PS_A.tile([128, 512], F32, name="ps_proj", tag="acc")
            for pp in range(2):
                lhsT = wt[(q, pp)].rearrange(
                    "p (j m) -> p j m", j=2)[:, :, mq * 128:(mq + 1) * 128]
                nc.tensor.matmul(ps[:], lhsT=lhsT, rhs=xTp[pp],
                                 start=(pp == 0), stop=(pp == 1),
                                 perf_mode=DR)
            evac(m, ps)

    def evac_alt(dst):
        def f(m, ps):
            sl = dst[:, m * 512:(m + 1) * 512]
            if m % 2 == 0:
                nc.vector.tensor_copy(sl, ps[:])
            else:
                nc.scalar.copy(sl, ps[:])
        return f

    proj("wq", evac_alt(QT))
    # data-dependency gate: gpsimd waits for the end of the Q projection
    # before issuing its bulk prefetches
    gdum = P.tile([128, 1], F8, name="gdum")
    nc.gpsimd.tensor_copy(gdum[:], QT[:, 16383:16384])
    late_dmas()
    proj("wk", evac_alt(KT))

    # ---- gamma/beta broadcast rows -> gb
    for i in range(4):
        psg = PS_A.tile([128, 512], F32, name="psg", tag="acc")
        nc.tensor.matmul(psg[:], lhsT=ones128[0:1, :],
                         rhs=vrow[0:1, i * 512:(i + 1) * 512],
                         start=True, stop=True, tile_position=(0, 0))
        nc.scalar.copy(gb[:, i * 512:(i + 1) * 512], psg[:])

    # ---- attention: Q-stationary S tiles, Z via exp-accum, diag numerators.
    # Row/col enumeration within a block: r = c*128 + t_local.
    QT4 = QT.rearrange("p (c hc t) -> p c hc t", c=8, hc=4)
    KT4 = KT.rearrange("p (c hc t) -> p c hc t", c=8, hc=4)
    Bc3 = Bc.rearrange("p (c t) -> p c t", c=8)

    # Per-block work is emitted one block LATE on the tensor engine (numer
    # matmuls and the D chain), so the PE never drains waiting on the
    # trailing exps of the current block (a drained PE gets clock-gated by
    # the HAM and runs at half rate for ~3us after).
    msks = {}
    nmrs = {}

    def numer_mm(a, c):
        nc.tensor.matmul(nmrs[a][:, c:c + 1], lhsT=msks.pop((a, c))[:],
                         rhs=ones_c[:], start=True, stop=True,
                         skip_group_check=True)

    def d_chain(a):
        # D = numer * DS/Z -> transpose to rows 0..7 -> selector-matmul
        # broadcast (all reads at base partition 0; offset bases hang the HW)
        ts = slice(a * 128, (a + 1) * 128)
        nmr = nmrs.pop(a)
        zrt = SCR.tile([128, 8], F32, name="zrt", tag="zrt", bufs=2)
        nc.vector.reciprocal(zrt[:], Zt[:, a * 8:a * 8 + 8])
        nc.vector.tensor_scalar_mul(zrt[:], zrt[:], DS)
        dc = SCR.tile([128, 8], BF16, name="dc", tag="dc", bufs=2)
        nc.vector.tensor_mul(dc[:], nmr[:], zrt[:])
        dt_ps = PS_A.tile([128, 128], BF16, name="dt_ps", tag="acc")
        nc.tensor.transpose(dt_ps[0:8, :], dc[:], identb[:])
        dt_sb = SCR.tile([128, 128], BF16, name="dt_sb", tag="dts", bufs=2)
        nc.vector.tensor_copy(dt_sb[0:8, :], dt_ps[0:8, :])
        bc_ps = PS_S.tile([128, 1024], F32, name="bc_ps", tag="S")
        for c in range(8):
            nc.tensor.matmul(bc_ps[:, c * 128:(c + 1) * 128],
                             lhsT=sel8[0:8, c * 128:(c + 1) * 128],
                             rhs=dt_sb[0:8, :],
                             start=True, stop=True, skip_group_check=True)
        nc.vector.tensor_copy(Bc3[:, :, ts],
                              bc_ps.rearrange("p (c t) -> p c t", c=8))

    for a in range(NBLK):
        ts = slice(a * 128, (a + 1) * 128)
        nmrs[a] = PS_A.tile([128, 8], F32, name="nmr", tag="acc")
        for c in range(8):
            ps = PS_S.tile([128, 1024], F32, name="ps_s", tag="S")
            for half in range(2):
                for pp in range(2):
                    lhsT = QT4[:, c, 2 * pp:2 * pp + 2, ts]
                    rhs = KT4[:, 4 * half:4 * half + 4,
                              2 * pp:2 * pp + 2, ts].transpose([0, 2, 1, 3])
                    nc.tensor.matmul(
                        ps[:, half * 512:(half + 1) * 512],
                        lhsT=lhsT, rhs=rhs,
                        start=(pp == 0), stop=(pp == 1), perf_mode=DR)
            if a > 0:
                numer_mm(a - 1, c)
            junk = SCR.tile([128, 1024], BF16, name="junk", tag="junk", bufs=3)
            nc.scalar.activation(junk[:], ps[:], AF.Exp, scale=ESC,
                                 accum_out=Zt[:, a * 8 + c:a * 8 + c + 1])
            msk = SCR.tile([128, 128], BF16, name="msk", tag="msk", bufs=10)
            nc.vector.tensor_mul(msk[:], junk[:, c * 128:(c + 1) * 128],
                                 identb[:])
            msks[(a, c)] = msk
        if a > 0:
            d_chain(a - 1)
    for c in range(8):
        numer_mm(NBLK - 1, c)
    d_chain(NBLK - 1)

    # ---- V projection fused with diag-softmax scaling -> hhT,
    # then attn out = hhT @ Wo ; + residual ; LN1 core.
    # Chunk pairs share one 2-bank psum so the evac-mult runs at [128,1024]
    # granularity (halves the DVE per-op overhead).
    hhT3 = hhT.rearrange("p (m t) -> p m t", t=512)
    wvsrc = d["wv"].rearrange("(t p) f -> t p f", p=128)
    wvt = {}
    for q in range(4):
        for pp in range(2):
            w = PW.tile([128, 2048], F8, name=f"w_wv{q}{pp}", tag="w")
            nc.sync.dma_start(w[:], wvsrc[pp * 4 + q])
            wvt[(q, pp)] = w
    for mp in range(16):
        ps = PS_S.tile([128, 1024], F32, name="ps_v", tag="S")
        for k in range(2):
            m = 2 * mp + k
            q, mq = m // 8, m % 8
            for pp in range(2):
                lhsT = wvt[(q, pp)].rearrange(
                    "p (j m) -> p j m", j=2)[:, :, mq * 128:(mq + 1) * 128]
                nc.tensor.matmul(ps[:, k * 512:(k + 1) * 512], lhsT=lhsT,
                                 rhs=xTp[pp], start=(pp == 0), stop=(pp == 1),
                                 perf_mode=DR)
        bsl = Bc3[:, mp // 2:mp // 2 + 1, :].broadcast_to((128, 2, 512))
        nc.vector.tensor_mul(
            hhT[:, mp * 1024:(mp + 1) * 1024].rearrange(
                "p (k t) -> p k t", k=2),
            ps.rearrange("p (k t) -> p k t", k=2), bsl)

    def ln_core(v_ap, out_ap):
        # mean via scalar Copy+accum (keeps the DVE free for the evac mults)
        nmu = ST.tile([128, 1], F32, name="nmu", tag="nmu")
        junkm = SCR.tile([128, 512], BF16, name="junkm", tag="junkf", bufs=2)
        nc.scalar.activation(junkm[:], v_ap, AF.Copy, accum_out=nmu[:])
        nc.vector.tensor_scalar_mul(nmu[:], nmu[:], -1.0 / H)
        ssq = ST.tile([128, 1], F32, name="ssq", tag="ssq")
        junkf = SCR.tile([128, 512], BF16, name="junkf", tag="junkf", bufs=2)
        nc.scalar.activation(junkf[:], v_ap, AF.Square, bias=nmu[:],
                             accum_out=ssq[:])
        sd = ST.tile([128, 1], F32, name="sd", tag="sd")
        nc.scalar.activation(sd[:], ssq[:], AF.Sqrt, scale=1.0 / H,
                             bias=epsc[:])
        rs = ST.tile([128, 1], F32, name="rs", tag="rs")
        nc.vector.reciprocal(rs[:], sd[:])
        nc.vector.tensor_scalar(out_ap, v_ap, nmu[:], rs[:],
                                op0=ALU.add, op1=ALU.mult)

    def ln(v_ap, gofs, bofs, out_ap):
        ln_core(v_ap, out_ap)
        nc.vector.tensor_mul(out_ap, out_ap, gb[:, gofs * 512:(gofs + 1) * 512])
        nc.vector.tensor_add(out_ap, out_ap, gb[:, bofs * 512:(bofs + 1) * 512])

    # Wo chains interleaved with the xcr->hh1T transposes (fp32, for FFN1)
    # so the PE never drains waiting on a trailing LN1 chain.
    wos3 = wos.rearrange("p (i j h) -> p i j h", i=16, j=2)

    def wo_chain(mt):
        ps_o = PS_A.tile([128, 512], F32, name="ps_o", tag="acc")
        for i in range(16):
            nc.tensor.matmul(
                ps_o[:], lhsT=hhT3[:, 2 * i:2 * i + 2, mt * 128:(mt + 1) * 128],
                rhs=wos3[:, i], start=(i == 0), stop=(i == 15), perf_mode=DR)
        v1 = SCR.tile([128, 512], F32, name="v1", tag="xr")
        nc.vector.scalar_tensor_tensor(
            out=v1[:], in0=ps_o[:], scalar=ODESC,
            in1=xrs[:, mt * 512:(mt + 1) * 512], op0=ALU.mult, op1=ALU.add)
        ln_core(v1[:], xcr[:, mt * 512:(mt + 1) * 512])

    def xcr_transpose(mt):
        for jj in range(4):
            tp = PS_A.tile([128, 128], F32, name="tp_h", tag="acc")
            nc.tensor.transpose(
                tp[:], xcr[:, mt * 512 + jj * 128:mt * 512 + jj * 128 + 128],
                identf[:])
            dst = hh1T[:, jj * 512 + mt * 128:jj * 512 + mt * 128 + 128]
            if jj % 2 == 0:
                nc.vector.tensor_copy(dst, tp[:])
            else:
                nc.scalar.copy(dst, tp[:])

    # pre-load the sqrt table set while the V/Wo matmuls run, so the first
    # LN1 chain doesn't eat the ~2.7us ACT_TABLE_LOAD serially
    sdum = ST.tile([128, 1], F32, name="sdum", tag="sd")
    nc.scalar.activation(sdum[:], epsc[:], AF.Sqrt)

    wo_chain(0)
    wo_chain(1)
    xcr_transpose(0)
    wo_chain(2)
    xcr_transpose(1)
    wo_chain(3)
    xcr_transpose(2)
    xcr_transpose(3)

    # ---- FFN1 (bf16): a1T = relu(Wf1'^T @ hh1T + bf1')
    for mf in range(16):
        ps = PS_A.tile([128, 512], F32, name="ps_f1", tag="acc")
        for hc in range(4):
            nc.tensor.matmul(
                ps[:], lhsT=wf1s[:, hc * 2048 + mf * 128:hc * 2048 + mf * 128 + 128],
                rhs=hh1T[:, hc * 512:(hc + 1) * 512],
                start=(hc == 0), stop=(hc == 3))
        nc.scalar.activation(a1T[:, mf * 512:(mf + 1) * 512], ps[:], AF.Relu,
                             bias=bf1c[:, mf:mf + 1])

    # hh1r = xcr*g1 + (beta1+bf2): emitted here so it lands in the FFN1
    # window where the DVE is otherwise idle (it's only read by FFN2's s2)
    for mt in range(4):
        sl = slice(mt * 512, (mt + 1) * 512)
        nc.vector.tensor_mul(hh1r[:, sl], xcr[:, sl], gb[:, 0:512])
        nc.vector.tensor_add(hh1r[:, sl], hh1r[:, sl], gb[:, 512:1024])

    # ---- FFN2 (bf16) + residual + LN2 -> out
    for mt in range(4):
        ps = PS_A.tile([128, 512], F32, name="ps_f2", tag="acc")
        for fc in range(16):
            nc.tensor.matmul(
                ps[:],
                lhsT=a1T[:, fc * 512 + mt * 128:fc * 512 + mt * 128 + 128],
                rhs=wf2s[:, fc * 512:(fc + 1) * 512],
                start=(fc == 0), stop=(fc == 15))
        s2 = SCR.tile([128, 512], F32, name="s2", tag="xr")
        nc.vector.tensor_add(s2[:], ps[:], hh1r[:, mt * 512:(mt + 1) * 512])
        outt = SCR.tile([128, 512], F32, name="outt", tag="xr")
        ln(s2[:], 2, 3, outt[:])
        nc.sync.dma_start(d["out"][mt * 128:(mt + 1) * 128, :], outt[:])

    for pool in (PS_B, PS_A, PS_S, ST, SCR, PW, P):
        pool.release()


def build(loop_n=None):
    nc = bacc.Bacc("TRN2", target_bir_lowering=False)
    d = {
        "xT": nc.dram_tensor("xT", (TPC, H), F8, kind="ExternalInput").ap(),
        "xr": nc.dram_tensor("xr", (TPC, H), F32, kind="ExternalInput").ap(),
        "wq": nc.dram_tensor("wq", (1024, 2048), F8, kind="ExternalInput").ap(),
        "wk": nc.dram_tensor("wk", (1024, 2048), F8, kind="ExternalInput").ap(),
        "wv": nc.dram_tensor("wv", (1024, 2048), F8, kind="ExternalInput").ap(),
        "wo": nc.dram_tensor("wo", (2048, 1024), F8, kind="ExternalInput").ap(),
        "wf1": nc.dram_tensor("wf1", (H, 4 * H), BF16, kind="ExternalInput").ap(),
        "wf2": nc.dram_tensor("wf2", (4 * H, H), BF16, kind="ExternalInput").ap(),
        "bf1": nc.dram_tensor("bf1", (4 * H,), F32, kind="ExternalInput").ap(),
        "vecs": nc.dram_tensor("vecs", (4, H), BF16, kind="ExternalInput").ap(),
        "ident": nc.dram_tensor("ident", (128, 128), F32,
                                kind="ExternalInput").ap(),
        "sel8": nc.dram_tensor("sel8", (8, 8 * 128), BF16,
                               kind="ExternalInput").ap(),
        "out": nc.dram_tensor("out", (TPC, H), F32, kind="ExternalOutput").ap(),
    }
    with tile.TileContext(nc) as tc:
        if loop_n is None:
            _emit(nc, tc, d)
        else:
            with tc.For_i(0, loop_n, 1):
                _emit(nc, tc, d)
    nc.finalize()
    return nc


def _pack_w(W):
    # tile-major fp8 packing for DoubleRow projections:
    # tile (pair, q): [p, j*1024+m] = W[(2*pair+j)*128+p, q*1024+m]
    W5 = np.asarray(W, np.float32).reshape(2, 2, 128, 4, 1024)
    return np.ascontiguousarray(
        W5.transpose(0, 3, 2, 1, 4).reshape(8 * 128, 2048)).astype(_F8)


def _pack_wo(W):
    # tile i: [p, j*512+h] = Wo[(2*i+j)*128+p, h]
    W4 = np.asarray(W, np.float32).reshape(16, 2, 128, 512)
    return np.ascontiguousarray(
        W4.transpose(0, 2, 1, 3).reshape(16 * 128, 1024)).astype(_F8)


def make_in_maps(inputs):
    xf = np.ascontiguousarray(
        np.asarray(inputs["x"], np.float32).reshape(TOK, H))
    g1 = np.asarray(inputs["g1"], np.float32)
    wf1 = np.asarray(inputs["Wf1"], np.float32)
    shared = {
        "wq": _pack_w(WS * np.asarray(inputs["Wq"], np.float32)),
        "wk": _pack_w(WS * np.asarray(inputs["Wk"], np.float32)),
        "wv": _pack_w(WS * np.asarray(inputs["Wv"], np.float32)),
        "wo": _pack_wo(WS * np.asarray(inputs["Wo"], np.float32)),
        "wf1": (g1[:, None] * wf1).astype(_BF),
        "wf2": np.asarray(inputs["Wf2"], np.float32).astype(_BF),
        "bf1": (np.asarray(inputs["bf1"], np.float32)
                + np.asarray(inputs["beta1"], np.float32) @ wf1),
        "vecs": np.ascontiguousarray(np.stack([
            g1,
            np.asarray(inputs["beta1"], np.float32)
            + np.asarray(inputs["bf2"], np.float32),
            np.asarray(inputs["g2"], np.float32),
            np.asarray(inputs["beta2"], np.float32)]).astype(_BF)),
        "ident": np.eye(128, dtype=np.float32),
        "sel8": np.ascontiguousarray(
            np.kron(np.eye(8, dtype=np.float32), np.ones((1, 128), np.float32))
            .astype(_BF)),
    }
    in_maps = []
    for c in range(NCORES):
        xs = xf[c * TPC:(c + 1) * TPC]
        m = dict(shared)
        m["xT"] = np.ascontiguousarray(xs.T).astype(_F8)
        m["xr"] = np.ascontiguousarray(xs)
        in_maps.append(m)
    return in_maps


_nc_cache = None


def _get_nc():
    global _nc_cache
    if _nc_cache is None:
        _nc_cache = build()
    return _nc_cache


def kernel(**inputs):
    nc = _get_nc()
    in_maps = make_in_maps(inputs)
    res = bass_utils.run_bass_kernel_spmd(nc, in_maps,
                                          core_ids=list(range(NCORES)))
    out = np.concatenate([r["out"] for r in res.results], axis=0)
    return out.reshape(B, T, H)


if __name__ == "__main__":
    nc = build()
    n_inst = sum(len(bb.instructions) for bb in nc.main_func.blocks)
    print("built OK; instructions:", n_inst)


# revision 35
# speedup vs baseline: 1.2355x; 1.0199x over previous
"""Trainium2 Bass kernel for the MultiHeadAttention-variant transformer block.

Math notes (derived from the module semantics):
  - The einsum 'batt,bath->bath' uses only the DIAGONAL of the softmax'd
    attention matrix, so per flat row i the attention output is
    softmax_diag_i * V[i].
  - The raw reshape (B,T,N*H)->(B,N,T,H) makes attention "blocks" couple only
    groups of 128 consecutive tokens (T/N = 1024/8 = 128); a block's 1024
    logical rows are the (chunk c, token t) pairs of those 128 tokens.
  => The whole layer is data-parallel over 128-token groups. We shard the
     4096 flattened tokens as 512 consecutive tokens per core (8 cores), with
     zero cross-core communication.

v2 design (vs the v1 baseline):
  - S is computed Q-STATIONARY with rows enumerated r = c*128 + t, so each
    row-tile of the 1024x1024 block-attention matrix is one out-psum
    [128, 1024] whose FREE axis spans the full softmax denominator.  The
    row sums Z then come for free from the Exp activation's accum_out, and
    the numerators are the diagonals of the c-th 128-col group (identity
    mask * ones-column matmul).  This removes the v1 Z-row matmul chains,
    the [1,512] reciprocals (28us of DVE!), and most broadcast plumbing.
  - All five weight matmuls (QKV, Wo, FFN1, FFN2) run fp8e4m3 DoubleRow.
    Weights are pre-scaled by 16 on the host (values ~0.02*N(0,1) would
    otherwise sit at the bottom of the fp8 range); the scales are folded
    into the exp() activation scale and the two residual descales.
  - LDWEIGHTS is fully hidden behind 512-col DR matmuls (216ns cadence),
    so the tensor-engine floor is the streamed column count.
"""

import sys

sys.path.insert(0, "/opt/trn_rl_repo")

import numpy as np
import ml_dtypes

import concourse.bass as bass
import concourse.mybir as mybir
import concourse.tile as tile
from concourse import bacc, bass_utils

F32 = mybir.dt.float32
BF16 = mybir.dt.bfloat16
F8 = mybir.dt.float8e4
AF = mybir.ActivationFunctionType
ALU = mybir.AluOpType
AX = mybir.AxisListType
DR = mybir.MatmulPerfMode.DoubleRow

H = 512
NH = 8
B = 4
T = 1024
TOK = B * T
NCORES = 8
TPC = TOK // NCORES  # 512 tokens per core
NBLK = TPC // 128  # 4 attention blocks per core
SCALE = float(1.0 / np.sqrt(H))
LN_EPS = 1e-5

_BF = ml_dtypes.bfloat16
_F8 = ml_dtypes.float8_e4m3

WS = 16.0  # host pre-scale on every weight matrix (fp8 range)
DS = 64.0  # extra scale on the softmax-diag D (fp8 range of hhT)
ESC = SCALE / (WS * WS)  # exp() input scale: undo Wq*16 * Wk*16
ODESC = 1.0 / (WS * WS * DS)  # Wo-path descale: V*16, Wo*16, D*64
FDESC = 1.0 / WS  # FFN descales


def _emit(nc, tc, d):
    """Emit the per-core program. d: dict of DRAM APs."""
    P = tc.alloc_tile_pool(name="persist", bufs=1)
    PW = tc.alloc_tile_pool(name="wpool", bufs=11)
    SCR = tc.alloc_tile_pool(name="scr", bufs=4)
    ST = tc.alloc_tile_pool(name="stats", bufs=4)
    PS_S = tc.alloc_tile_pool(name="ps_s", bufs=2, space="PSUM")
    PS_A = tc.alloc_tile_pool(name="ps_a", bufs=4, space="PSUM")

    # ---- persistent tiles
    xT = P.tile([128, 4 * 512], F8, name="xT")  # x^T, 4 h-chunks
    identf = P.tile([128, 128], F32, name="identf")
    identb = P.tile([128, 128], BF16, name="identb")
    ones_c = P.tile([128, 1], BF16, name="ones_c")  # rhs for diag-col matmul
    ones128 = P.tile([128, 128], BF16, name="ones128")  # bcast lhsT rows
    vrow = P.tile([1, 4 * 512], BF16, name="vrow")  # g1,vsum,g2,b2 rows
    bf1c = P.tile([128, 16], F32, name="bf1c")  # bf1 column-major
    gb = P.tile([128, 4 * 512], F32, name="gb")  # bcast g1,vsum,g2,b2
    Bc = P.tile([128, 8 * 512], BF16, name="Bc")  # D*DS bcast, [p, c*512+t]
    Zt = P.tile([128, 32], F32, name="Zt")  # softmax denoms, col a*8+c
    epsc = P.tile([128, 1], F32, name="epsc")
    xrs = P.tile([128, 4 * 512], F32, name="xrs")  # residual x, [p, mt*512+h]
    sel8 = P.tile([128, 8 * 128], BF16, name="sel8")  # sel8[k,c*128+p]=(k==c)
    QT = P.tile([128, 32 * 512], F8, name="QT")
    KT = P.tile([128, 32 * 512], F8, name="KT")
    hhT = P.tile([128, 32 * 512], F8, name="hhT")
    xcr = P.tile([128, 4 * 512], F32, name="xcr")  # LN1 core out (no g/b)
    hh1T = P.tile([128, 4 * 512], BF16, name="hh1T")  # xcr^T for FFN1
    hh1r = P.tile([128, 4 * 512], F32, name="hh1r")  # true hh1 residual
    a1T = P.tile([128, 16 * 512], BF16, name="a1T")  # relu acts, [p, fc*512+t]
    wos = P.tile([128, 16 * 1024], F8, name="wos")  # Wo DR-packed
    wf1s = P.tile([128, 4 * 2048], BF16, name="wf1s")  # [p, hc*2048+f]
    wf2s = P.tile([128, 16 * 512], BF16, name="wf2s")  # [p, fc*512+h]

    # ---- input DMAs.  sync ring: critical path (x, ident, wq/wk/wv stream).
    # scalar ring: everything needed later (vecs, bf1, xr, wo, wf1, wf2).
    nc.sync.dma_start(
        xT.rearrange("p (hc t) -> p hc t", hc=4),
        d["xT"].rearrange("(hc p) t -> p hc t", p=128))
    nc.sync.dma_start(identf[:], d["ident"][:])
    nc.scalar.dma_start(vrow[0:1, :], d["vecs"].rearrange("v h -> (v h)")[None, :])
    nc.scalar.dma_start(bf1c[:], d["bf1"].rearrange("(m p) -> p m", p=128))

    nc.vector.memset(ones_c[:], 1.0)
    nc.vector.memset(ones128[:], 1.0)
    nc.vector.memset(epsc[:], LN_EPS)
    nc.vector.tensor_copy(identb[:], identf[:])

    def late_dmas():
        # big late-deadline weights go on the (otherwise idle) gpsimd SWDGE
        # ring, gated behind the QK weight stream (emitted mid-phase-A) so
        # their 7MB doesn't steal fabric bandwidth from the critical path
        nc.gpsimd.dma_start(sel8[0:8, :], d["sel8"][:])
        nc.gpsimd.dma_start(
            xrs.rearrange("p (mt h) -> p mt h", mt=4),
            d["xr"].rearrange("(mt p) h -> p mt h", p=128))
        nc.gpsimd.dma_start(
            wos.rearrange("p (i f) -> p i f", i=16),
            d["wo"].rearrange("(i p) f -> p i f", p=128))
        nc.gpsimd.dma_start(
            wf1s.rearrange("p (hc f) -> p hc f", hc=4),
            d["wf1"].rearrange("(hc p) f -> p hc f", p=128))
        nc.gpsimd.dma_start(
            wf2s.rearrange("p (fc h) -> p fc h", fc=16),
            d["wf2"].rearrange("(fc p) h -> p fc h", p=128))

    # ---- projections (fp8 DoubleRow): dst^T = W^T @ x^T.  Weight DRAM is
    # pre-packed tile-major: tile (pair, q) rows,
    # [p, j*1024+m] = W[(2*pair+j)*128+p, q*1024+m].
    xTp = [xT[:, pp * 1024:(pp + 1) * 1024].rearrange("p (j t) -> p j t", j=2)
           for pp in range(2)]

    def proj(wname, evac):
        wsrc = d[wname].rearrange("(t p) f -> t p f", p=128)
        wt = {}
        for q in range(4):
            for pp in range(2):
                w = PW.tile([128, 2048], F8, name=f"w_{wname}{q}{pp}", tag="w")
                nc.sync.dma_start(w[:], wsrc[pp * 4 + q])
                wt[(q, pp)] = w
        for m in range(32):
            q, mq = m // 8, m % 8
            ps = PS_A.tile([128, 512], F32, name="ps_proj", tag="acc")
            for pp in range(2):
                lhsT = wt[(q, pp)].rearrange(
                    "p (j m) -> p j m", j=2)[:, :, mq * 128:(mq + 1) * 128]
                nc.tensor.matmul(ps[:], lhsT=lhsT, rhs=xTp[pp],
                                 start=(pp == 0), stop=(pp == 1),
                                 perf_mode=DR)
            evac(m, ps)

    def evac_alt(dst):
        def f(m, ps):
            sl = dst[:, m * 512:(m + 1) * 512]
            if m % 2 == 0:
                nc.vector.tensor_copy(sl, ps[:])
            else:
                nc.scalar.copy(sl, ps[:])
        return f

    proj("wk", evac_alt(KT))
    # data-dependency gate: gpsimd waits for the end of the K projection
    # before issuing its bulk prefetches
    gdum = P.tile([128, 1], F8, name="gdum")
    nc.gpsimd.tensor_copy(gdum[:], KT[:, 16383:16384])
    late_dmas()

    # ---- gamma/beta broadcast rows -> gb
    for i in range(4):
        psg = PS_A.tile([128, 512], F32, name="psg", tag="acc")
        nc.tensor.matmul(psg[:], lhsT=ones128[0:1, :],
                         rhs=vrow[0:1, i * 512:(i + 1) * 512],
                         start=True, stop=True, tile_position=(0, 0))
        nc.scalar.copy(gb[:, i * 512:(i + 1) * 512], psg[:])

    # ---- attention: Q-stationary S tiles, Z via exp-accum, diag numerators.
    # Row/col enumeration within a block: r = c*128 + t_local.
    QT4 = QT.rearrange("p (c hc t) -> p c hc t", c=8, hc=4)
    KT4 = KT.rearrange("p (c hc t) -> p c hc t", c=8, hc=4)
    Bc3 = Bc.rearrange("p (c t) -> p c t", c=8)

    # Per-block work is emitted one block LATE on the tensor engine (numer
    # matmuls and the D chain), so the PE never drains waiting on the
    # trailing exps of the current block (a drained PE gets clock-gated by
    # the HAM and runs at half rate for ~3us after).
    msks = {}
    nmr = PS_A.tile([128, 32], F32, name="nmr", tag="acc")  # col = a*8+c

    def numer_mm(a, c):
        nc.tensor.matmul(nmr[:, a * 8 + c:a * 8 + c + 1],
                         lhsT=msks.pop((a, c))[:],
                         rhs=ones_c[:], start=True, stop=True,
                         skip_group_check=True)

    def d_chain(a):
        # D = numer * DS/Z -> transpose to rows 0..7 -> selector-matmul
        # broadcast (all reads at base partition 0; offset bases hang the HW)
        ts = slice(a * 128, (a + 1) * 128)
        zrt = SCR.tile([128, 8], F32, name="zrt", tag="zrt", bufs=2)
        nc.vector.reciprocal(zrt[:], Zt[:, a * 8:a * 8 + 8])
        nc.vector.tensor_scalar_mul(zrt[:], zrt[:], DS)
        dc = SCR.tile([128, 8], BF16, name="dc", tag="dc", bufs=2)
        nc.vector.tensor_mul(dc[:], nmr[:, a * 8:a * 8 + 8], zrt[:])
        dt_ps = PS_A.tile([128, 128], BF16, name="dt_ps", tag="acc")
        nc.tensor.transpose(dt_ps[0:8, :], dc[:], identb[:])
        dt_sb = SCR.tile([128, 128], BF16, name="dt_sb", tag="dts", bufs=2)
        nc.vector.tensor_copy(dt_sb[0:8, :], dt_ps[0:8, :])
        bc_ps = PS_S.tile([128, 1024], F32, name="bc_ps", tag="S")
        for c in range(8):
            nc.tensor.matmul(bc_ps[:, c * 128:(c + 1) * 128],
                             lhsT=sel8[0:8, c * 128:(c + 1) * 128],
                             rhs=dt_sb[0:8, :],
                             start=True, stop=True, skip_group_check=True)
        nc.vector.tensor_copy(Bc3[:, :, ts],
                              bc_ps.rearrange("p (c t) -> p c t", c=8))

    # Merged Q-projection + attention: Q chunks for column-group c1 are
    # produced right before the four S groups that consume them, so the
    # exp stream starts ~30us earlier than a separate Q phase would allow.
    wqsrc = d["wq"].rearrange("(t p) f -> t p f", p=128)
    wqt = {}
    for q in range(4):
        for pp in range(2):
            w = PW.tile([128, 2048], F8, name=f"w_wq{q}{pp}", tag="w")
            nc.sync.dma_start(w[:], wqsrc[pp * 4 + q])
            wqt[(q, pp)] = w
    pend = []
    for c1 in range(8):
        for k in range(4):
            m = 4 * c1 + k
            q, mq = m // 8, m % 8
            psq = PS_A.tile([128, 512], F32, name="psq", tag="acc")
            for pp in range(2):
                lhsT = wqt[(q, pp)].rearrange(
                    "p (j m) -> p j m", j=2)[:, :, mq * 128:(mq + 1) * 128]
                nc.tensor.matmul(psq[:], lhsT=lhsT, rhs=xTp[pp],
                                 start=(pp == 0), stop=(pp == 1),
                                 perf_mode=DR)
            nc.vector.tensor_copy(QT[:, m * 512:(m + 1) * 512], psq[:])
        for a in range(NBLK):
            ts = slice(a * 128, (a + 1) * 128)
            ps = PS_S.tile([128, 1024], F32, name="ps_s", tag="S")
            for half in range(2):
                for pp in range(2):
                    lhsT = QT4[:, c1, 2 * pp:2 * pp + 2, ts]
                    rhs = KT4[:, 4 * half:4 * half + 4,
                              2 * pp:2 * pp + 2, ts].transpose([0, 2, 1, 3])
                    nc.tensor.matmul(
                        ps[:, half * 512:(half + 1) * 512],
                        lhsT=lhsT, rhs=rhs,
                        start=(pp == 0), stop=(pp == 1), perf_mode=DR)
            if len(pend) >= 8:
                pend.pop(0)()
            junk = SCR.tile([128, 1024], BF16, name="junk", tag="junk", bufs=3)
            nc.scalar.activation(junk[:], ps[:], AF.Exp, scale=ESC,
                                 accum_out=Zt[:, a * 8 + c1:a * 8 + c1 + 1])
            msk = SCR.tile([128, 128], BF16, name="msk", tag="msk", bufs=12)
            nc.vector.tensor_mul(msk[:], junk[:, c1 * 128:(c1 + 1) * 128],
                                 identb[:])
            msks[(a, c1)] = msk
            pend.append(lambda a=a, c1=c1: numer_mm(a, c1))
    for f in pend:
        f()
    for a in range(NBLK):
        d_chain(a)

    # ---- V projection fused with diag-softmax scaling -> hhT,
    # then attn out = hhT @ Wo ; + residual ; LN1 core.
    # Chunk pairs share one 2-bank psum so the evac-mult runs at [128,1024]
    # granularity (halves the DVE per-op overhead).
    hhT3 = hhT.rearrange("p (m t) -> p m t", t=512)
    wvsrc = d["wv"].rearrange("(t p) f -> t p f", p=128)
    wvt = {}
    for q in range(4):
        for pp in range(2):
            w = PW.tile([128, 2048], F8, name=f"w_wv{q}{pp}", tag="w")
            nc.sync.dma_start(w[:], wvsrc[pp * 4 + q])
            wvt[(q, pp)] = w
    for mp in range(16):
        ps = PS_S.tile([128, 1024], F32, name="ps_v", tag="S")
        for k in range(2):
            m = 2 * mp + k
            q, mq = m // 8, m % 8
            for pp in range(2):
                lhsT = wvt[(q, pp)].rearrange(
                    "p (j m) -> p j m", j=2)[:, :, mq * 128:(mq + 1) * 128]
                nc.tensor.matmul(ps[:, k * 512:(k + 1) * 512], lhsT=lhsT,
                                 rhs=xTp[pp], start=(pp == 0), stop=(pp == 1),
                                 perf_mode=DR)
        bsl = Bc3[:, mp // 2:mp // 2 + 1, :].broadcast_to((128, 2, 512))
        nc.vector.tensor_mul(
            hhT[:, mp * 1024:(mp + 1) * 1024].rearrange(
                "p (k t) -> p k t", k=2),
            ps.rearrange("p (k t) -> p k t", k=2), bsl)

    def ln_core(v_ap, out_ap):
        # mean via scalar Copy+accum (keeps the DVE free for the evac mults)
        nmu = ST.tile([128, 1], F32, name="nmu", tag="nmu")
        junkm = SCR.tile([128, 512], BF16, name="junkm", tag="junkf", bufs=2)
        nc.scalar.activation(junkm[:], v_ap, AF.Copy, accum_out=nmu[:])
        nc.vector.tensor_scalar_mul(nmu[:], nmu[:], -1.0 / H)
        ssq = ST.tile([128, 1], F32, name="ssq", tag="ssq")
        junkf = SCR.tile([128, 512], BF16, name="junkf", tag="junkf", bufs=2)
        nc.scalar.activation(junkf[:], v_ap, AF.Square, bias=nmu[:],
                             accum_out=ssq[:])
        sd = ST.tile([128, 1], F32, name="sd", tag="sd")
        nc.scalar.activation(sd[:], ssq[:], AF.Sqrt, scale=1.0 / H,
                             bias=epsc[:])
        rs = ST.tile([128, 1], F32, name="rs", tag="rs")
        nc.vector.reciprocal(rs[:], sd[:])
        nc.vector.tensor_scalar(out_ap, v_ap, nmu[:], rs[:],
                                op0=ALU.add, op1=ALU.mult)

    def ln(v_ap, gofs, bofs, out_ap):
        ln_core(v_ap, out_ap)
        nc.vector.tensor_mul(out_ap, out_ap, gb[:, gofs * 512:(gofs + 1) * 512])
        nc.vector.tensor_add(out_ap, out_ap, gb[:, bofs * 512:(bofs + 1) * 512])

    # Wo chains interleaved with the xcr->hh1T transposes (fp32, for FFN1)
    # so the PE never drains waiting on a trailing LN1 chain.
    wos3 = wos.rearrange("p (i j h) -> p i j h", i=16, j=2)

    def wo_chain(mt):
        ps_o = PS_A.tile([128, 512], F32, name="ps_o", tag="acc")
        for i in range(16):
            nc.tensor.matmul(
                ps_o[:], lhsT=hhT3[:, 2 * i:2 * i + 2, mt * 128:(mt + 1) * 128],
                rhs=wos3[:, i], start=(i == 0), stop=(i == 15), perf_mode=DR)
        v1 = SCR.tile([128, 512], F32, name="v1", tag="xr")
        nc.vector.scalar_tensor_tensor(
            out=v1[:], in0=ps_o[:], scalar=ODESC,
            in1=xrs[:, mt * 512:(mt + 1) * 512], op0=ALU.mult, op1=ALU.add)
        ln_core(v1[:], xcr[:, mt * 512:(mt + 1) * 512])

    def xcr_transpose(mt):
        for jj in range(4):
            tp = PS_A.tile([128, 128], F32, name="tp_h", tag="acc")
            nc.tensor.transpose(
                tp[:], xcr[:, mt * 512 + jj * 128:mt * 512 + jj * 128 + 128],
                identf[:])
            dst = hh1T[:, jj * 512 + mt * 128:jj * 512 + mt * 128 + 128]
            if jj % 2 == 0:
                nc.vector.tensor_copy(dst, tp[:])
            else:
                nc.scalar.copy(dst, tp[:])

    # pre-load the sqrt table set while the V/Wo matmuls run, so the first
    # LN1 chain doesn't eat the ~2.7us ACT_TABLE_LOAD serially
    sdum = ST.tile([128, 1], F32, name="sdum", tag="sd")
    nc.scalar.activation(sdum[:], epsc[:], AF.Sqrt)

    wo_chain(0)
    wo_chain(1)
    xcr_transpose(0)
    wo_chain(2)
    xcr_transpose(1)
    wo_chain(3)
    xcr_transpose(2)
    xcr_transpose(3)

    # ---- FFN1 (bf16): a1T = relu(Wf1'^T @ hh1T + bf1')
    for mf in range(16):
        ps = PS_A.tile([128, 512], F32, name="ps_f1", tag="acc")
        for hc in range(4):
            nc.tensor.matmul(
                ps[:], lhsT=wf1s[:, hc * 2048 + mf * 128:hc * 2048 + mf * 128 + 128],
                rhs=hh1T[:, hc * 512:(hc + 1) * 512],
                start=(hc == 0), stop=(hc == 3))
        nc.scalar.activation(a1T[:, mf * 512:(mf + 1) * 512], ps[:], AF.Relu,
                             bias=bf1c[:, mf:mf + 1])

    # hh1r = xcr*g1 + (beta1+bf2): emitted here so it lands in the FFN1
    # window where the DVE is otherwise idle (it's only read by FFN2's s2)
    for mt in range(4):
        sl = slice(mt * 512, (mt + 1) * 512)
        nc.vector.tensor_mul(hh1r[:, sl], xcr[:, sl], gb[:, 0:512])
        nc.vector.tensor_add(hh1r[:, sl], hh1r[:, sl], gb[:, 512:1024])

    # ---- FFN2 (bf16) + residual + LN2 -> out
    for mt in range(4):
        ps = PS_A.tile([128, 512], F32, name="ps_f2", tag="acc")
        for fc in range(16):
            nc.tensor.matmul(
                ps[:],
                lhsT=a1T[:, fc * 512 + mt * 128:fc * 512 + mt * 128 + 128],
                rhs=wf2s[:, fc * 512:(fc + 1) * 512],
                start=(fc == 0), stop=(fc == 15))
        s2 = SCR.tile([128, 512], F32, name="s2", tag="xr")
        nc.vector.tensor_add(s2[:], ps[:], hh1r[:, mt * 512:(mt + 1) * 512])
        outt = SCR.tile([128, 512], F32, name="outt", tag="xr")
        ln(s2[:], 2, 3, outt[:])
        nc.sync.dma_start(d["out"][mt * 128:(mt + 1) * 128, :], outt[:])

    for pool in (PS_B, PS_A, PS_S, ST, SCR, PW, P):
        pool.release()


def build(loop_n=None):
    nc = bacc.Bacc("TRN2", target_bir_lowering=False)
    d = {
        "xT": nc.dram_tensor("xT", (TPC, H), F8, kind="ExternalInput").ap(),
        "xr": nc.dram_tensor("xr", (TPC, H), F32, kind="ExternalInput").ap(),
        "wq": nc.dram_tensor("wq", (1024, 2048), F8, kind="ExternalInput").ap(),
        "wk": nc.dram_tensor("wk", (1024, 2048), F8, kind="ExternalInput").ap(),
        "wv": nc.dram_tensor("wv", (1024, 2048), F8, kind="ExternalInput").ap(),
        "wo": nc.dram_tensor("wo", (2048, 1024), F8, kind="ExternalInput").ap(),
        "wf1": nc.dram_tensor("wf1", (H, 4 * H), BF16, kind="ExternalInput").ap(),
        "wf2": nc.dram_tensor("wf2", (4 * H, H), BF16, kind="ExternalInput").ap(),
        "bf1": nc.dram_tensor("bf1", (4 * H,), F32, kind="ExternalInput").ap(),
        "vecs": nc.dram_tensor("vecs", (4, H), BF16, kind="ExternalInput").ap(),
        "ident": nc.dram_tensor("ident", (128, 128), F32,
                                kind="ExternalInput").ap(),
        "sel8": nc.dram_tensor("sel8", (8, 8 * 128), BF16,
                               kind="ExternalInput").ap(),
        "out": nc.dram_tensor("out", (TPC, H), F32, kind="ExternalOutput").ap(),
    }
    with tile.TileContext(nc) as tc:
        if loop_n is None:
            _emit(nc, tc, d)
        else:
            with tc.For_i(0, loop_n, 1):
                _emit(nc, tc, d)
    nc.finalize()
    return nc


def _pack_w(W):
    # tile-major fp8 packing for DoubleRow projections:
    # tile (pair, q): [p, j*1024+m] = W[(2*pair+j)*128+p, q*1024+m]
    W5 = np.asarray(W, np.float32).reshape(2, 2, 128, 4, 1024)
    return np.ascontiguousarray(
        W5.transpose(0, 3, 2, 1, 4).reshape(8 * 128, 2048)).astype(_F8)


def _pack_wo(W):
    # tile i: [p, j*512+h] = Wo[(2*i+j)*128+p, h]
    W4 = np.asarray(W, np.float32).reshape(16, 2, 128, 512)
    return np.ascontiguousarray(
        W4.transpose(0, 2, 1, 3).reshape(16 * 128, 1024)).astype(_F8)


def make_in_maps(inputs):
    xf = np.ascontiguousarray(
        np.asarray(inputs["x"], np.float32).reshape(TOK, H))
    g1 = np.asarray(inputs["g1"], np.float32)
    wf1 = np.asarray(inputs["Wf1"], np.float32)
    shared = {
        "wq": _pack_w(WS * np.asarray(inputs["Wq"], np.float32)),
        "wk": _pack_w(WS * np.asarray(inputs["Wk"], np.float32)),
        "wv": _pack_w(WS * np.asarray(inputs["Wv"], np.float32)),
        "wo": _pack_wo(WS * np.asarray(inputs["Wo"], np.float32)),
        "wf1": (g1[:, None] * wf1).astype(_BF),
        "wf2": np.asarray(inputs["Wf2"], np.float32).astype(_BF),
        "bf1": (np.asarray(inputs["bf1"], np.float32)
                + np.asarray(inputs["beta1"], np.float32) @ wf1),
        "vecs": np.ascontiguousarray(np.stack([
            g1,
            np.asarray(inputs["beta1"], np.float32)
            + np.asarray(inputs["bf2"], np.float32),
            np.asarray(inputs["g2"], np.float32),
            np.asarray(inputs["beta2"], np.float32)]).astype(_BF)),
        "ident": np.eye(128, dtype=np.float32),
        "sel8": np.ascontiguousarray(
            np.kron(np.eye(8, dtype=np.float32), np.ones((1, 128), np.float32))
            .astype(_BF)),
    }
    in_maps = []
    for c in range(NCORES):
        xs = xf[c * TPC:(c + 1) * TPC]
        m = dict(shared)
        m["xT"] = np.ascontiguousarray(xs.T).astype(_F8)
        m["xr"] = np.ascontiguousarray(xs)
        in_maps.append(m)
    return in_maps


_nc_cache = None


def _get_nc():
    global _nc_cache
    if _nc_cache is None:
        _nc_cache = build()
    return _nc_cache


def kernel(**inputs):
    nc = _get_nc()
    in_maps = make_in_maps(inputs)
    res = bass_utils.run_bass_kernel_spmd(nc, in_maps,
                                          core_ids=list(range(NCORES)))
    out = np.concatenate([r["out"] for r in res.results], axis=0)
    return out.reshape(B, T, H)


if __name__ == "__main__":
    nc = build()
    n_inst = sum(len(bb.instructions) for bb in nc.main_func.blocks)
    print("built OK; instructions:", n_inst)


# revision 37
# speedup vs baseline: 1.2456x; 1.0081x over previous
"""Trainium2 Bass kernel for the MultiHeadAttention-variant transformer block.

Math notes (derived from the module semantics):
  - The einsum 'batt,bath->bath' uses only the DIAGONAL of the softmax'd
    attention matrix, so per flat row i the attention output is
    softmax_diag_i * V[i].
  - The raw reshape (B,T,N*H)->(B,N,T,H) makes attention "blocks" couple only
    groups of 128 consecutive tokens (T/N = 1024/8 = 128); a block's 1024
    logical rows are the (chunk c, token t) pairs of those 128 tokens.
  => The whole layer is data-parallel over 128-token groups. We shard the
     4096 flattened tokens as 512 consecutive tokens per core (8 cores), with
     zero cross-core communication.

v2 design (vs the v1 baseline):
  - S is computed Q-STATIONARY with rows enumerated r = c*128 + t, so each
    row-tile of the 1024x1024 block-attention matrix is one out-psum
    [128, 1024] whose FREE axis spans the full softmax denominator.  The
    row sums Z then come for free from the Exp activation's accum_out, and
    the numerators are the diagonals of the c-th 128-col group (identity
    mask * ones-column matmul).  This removes the v1 Z-row matmul chains,
    the [1,512] reciprocals (28us of DVE!), and most broadcast plumbing.
  - All five weight matmuls (QKV, Wo, FFN1, FFN2) run fp8e4m3 DoubleRow.
    Weights are pre-scaled by 16 on the host (values ~0.02*N(0,1) would
    otherwise sit at the bottom of the fp8 range); the scales are folded
    into the exp() activation scale and the two residual descales.
  - LDWEIGHTS is fully hidden behind 512-col DR matmuls (216ns cadence),
    so the tensor-engine floor is the streamed column count.
"""

import sys

sys.path.insert(0, "/opt/trn_rl_repo")

import numpy as np
import ml_dtypes

import concourse.bass as bass
import concourse.mybir as mybir
import concourse.tile as tile
from concourse import bacc, bass_utils

F32 = mybir.dt.float32
BF16 = mybir.dt.bfloat16
F8 = mybir.dt.float8e4
AF = mybir.ActivationFunctionType
ALU = mybir.AluOpType
AX = mybir.AxisListType
DR = mybir.MatmulPerfMode.DoubleRow

H = 512
NH = 8
B = 4
T = 1024
TOK = B * T
NCORES = 8
TPC = TOK // NCORES  # 512 tokens per core
NBLK = TPC // 128  # 4 attention blocks per core
SCALE = float(1.0 / np.sqrt(H))
LN_EPS = 1e-5

_BF = ml_dtypes.bfloat16
_F8 = ml_dtypes.float8_e4m3

WS = 16.0  # host pre-scale on every weight matrix (fp8 range)
DS = 64.0  # extra scale on the softmax-diag D (fp8 range of hhT)
ESC = SCALE / (WS * WS)  # exp() input scale: undo Wq*16 * Wk*16
ODESC = 1.0 / (WS * WS * DS)  # Wo-path descale: V*16, Wo*16, D*64
FDESC = 1.0 / WS  # FFN descales


def _emit(nc, tc, d):
    """Emit the per-core program. d: dict of DRAM APs."""
    P = tc.alloc_tile_pool(name="persist", bufs=1)
    PW = tc.alloc_tile_pool(name="wpool", bufs=16)
    SCR = tc.alloc_tile_pool(name="scr", bufs=4)
    ST = tc.alloc_tile_pool(name="stats", bufs=4)
    PS_S = tc.alloc_tile_pool(name="ps_s", bufs=2, space="PSUM")
    PS_A = tc.alloc_tile_pool(name="ps_a", bufs=4, space="PSUM")

    # ---- persistent tiles
    xT = P.tile([128, 4 * 512], F8, name="xT")  # x^T, 4 h-chunks
    identf = P.tile([128, 128], F32, name="identf")
    identb = P.tile([128, 128], BF16, name="identb")
    ones_c = P.tile([128, 1], BF16, name="ones_c")  # rhs for diag-col matmul
    ones128 = P.tile([128, 128], BF16, name="ones128")  # bcast lhsT rows
    vrow = P.tile([1, 4 * 512], BF16, name="vrow")  # g1,vsum,g2,b2 rows
    bf1c = P.tile([128, 16], F32, name="bf1c")  # bf1 column-major
    gb = P.tile([128, 4 * 512], BF16, name="gb")  # bcast g1,vsum,g2,b2
    Bc = P.tile([128, 8 * 512], F8, name="Bc")  # D*DS bcast, [p, c*512+t]
    Zt = P.tile([128, 32], F32, name="Zt")  # softmax denoms, col a*8+c
    epsc = P.tile([128, 1], F32, name="epsc")
    xrs = P.tile([128, 4 * 512], F32, name="xrs")  # residual x, [p, mt*512+h]
    sel8 = P.tile([128, 8 * 128], BF16, name="sel8")  # sel8[k,c*128+p]=(k==c)
    QT = P.tile([128, 32 * 512], F8, name="QT")
    KT = P.tile([128, 32 * 512], F8, name="KT")
    hhT = P.tile([128, 32 * 512], F8, name="hhT")
    xcr = P.tile([128, 4 * 512], F32, name="xcr")  # LN1 core out (no g/b)
    hh1T = P.tile([128, 4 * 512], BF16, name="hh1T")  # xcr^T for FFN1
    hh1r = P.tile([128, 4 * 512], F32, name="hh1r")  # true hh1 residual
    a1T = P.tile([128, 16 * 512], BF16, name="a1T")  # relu acts, [p, fc*512+t]
    wos = P.tile([128, 16 * 1024], F8, name="wos")  # Wo DR-packed
    wf1s = P.tile([128, 4 * 2048], BF16, name="wf1s")  # [p, hc*2048+f]
    wf2s = P.tile([128, 16 * 512], BF16, name="wf2s")  # [p, fc*512+h]

    # ---- input DMAs.  sync ring: critical path (x, ident, wq/wk/wv stream).
    # scalar ring: everything needed later (vecs, bf1, xr, wo, wf1, wf2).
    nc.sync.dma_start(
        xT.rearrange("p (hc t) -> p hc t", hc=4),
        d["xT"].rearrange("(hc p) t -> p hc t", p=128))
    nc.sync.dma_start(identf[:], d["ident"][:])
    nc.scalar.dma_start(vrow[0:1, :], d["vecs"].rearrange("v h -> (v h)")[None, :])
    nc.scalar.dma_start(bf1c[:], d["bf1"].rearrange("(m p) -> p m", p=128))

    nc.vector.memset(ones_c[:], 1.0)
    nc.vector.memset(ones128[:], 1.0)
    nc.vector.memset(epsc[:], LN_EPS)
    nc.vector.tensor_copy(identb[:], identf[:])

    def late_dmas():
        # big late-deadline weights go on the (otherwise idle) gpsimd SWDGE
        # ring, gated behind the QK weight stream (emitted mid-phase-A) so
        # their 7MB doesn't steal fabric bandwidth from the critical path
        nc.gpsimd.dma_start(sel8[0:8, :], d["sel8"][:])
        nc.gpsimd.dma_start(
            xrs.rearrange("p (mt h) -> p mt h", mt=4),
            d["xr"].rearrange("(mt p) h -> p mt h", p=128))
        nc.gpsimd.dma_start(
            wos.rearrange("p (i f) -> p i f", i=16),
            d["wo"].rearrange("(i p) f -> p i f", p=128))
        nc.gpsimd.dma_start(
            wf1s.rearrange("p (hc f) -> p hc f", hc=4),
            d["wf1"].rearrange("(hc p) f -> p hc f", p=128))
        nc.gpsimd.dma_start(
            wf2s.rearrange("p (fc h) -> p fc h", fc=16),
            d["wf2"].rearrange("(fc p) h -> p fc h", p=128))

    # ---- projections (fp8 DoubleRow): dst^T = W^T @ x^T.  Weight DRAM is
    # pre-packed tile-major: tile (pair, q) rows,
    # [p, j*1024+m] = W[(2*pair+j)*128+p, q*1024+m].
    xTp = [xT[:, pp * 1024:(pp + 1) * 1024].rearrange("p (j t) -> p j t", j=2)
           for pp in range(2)]

    def proj(wname, evac):
        wsrc = d[wname].rearrange("(t p) f -> t p f", p=128)
        wt = {}
        for q in range(4):
            for pp in range(2):
                w = PW.tile([128, 2048], F8, name=f"w_{wname}{q}{pp}", tag="w")
                nc.sync.dma_start(w[:], wsrc[pp * 4 + q])
                wt[(q, pp)] = w
        for m in range(32):
            q, mq = m // 8, m % 8
            ps = PS_A.tile([128, 512], F32, name="ps_proj", tag="acc")
            for pp in range(2):
                lhsT = wt[(q, pp)].rearrange(
                    "p (j m) -> p j m", j=2)[:, :, mq * 128:(mq + 1) * 128]
                nc.tensor.matmul(ps[:], lhsT=lhsT, rhs=xTp[pp],
                                 start=(pp == 0), stop=(pp == 1),
                                 perf_mode=DR)
            evac(m, ps)

    def evac_alt(dst):
        def f(m, ps):
            sl = dst[:, m * 512:(m + 1) * 512]
            if m % 2 == 0:
                nc.vector.tensor_copy(sl, ps[:])
            else:
                nc.scalar.copy(sl, ps[:])
        return f

    proj("wk", evac_alt(KT))
    # data-dependency gate: gpsimd waits for the end of the K projection
    # before issuing its bulk prefetches
    gdum = P.tile([128, 1], F8, name="gdum")
    nc.gpsimd.tensor_copy(gdum[:], KT[:, 16383:16384])
    late_dmas()

    # ---- attention: Q-stationary S tiles, Z via exp-accum, diag numerators.
    # Row/col enumeration within a block: r = c*128 + t_local.
    QT4 = QT.rearrange("p (c hc t) -> p c hc t", c=8, hc=4)
    KT4 = KT.rearrange("p (c hc t) -> p c hc t", c=8, hc=4)
    Bc3 = Bc.rearrange("p (c t) -> p c t", c=8)

    # Per-block work is emitted one block LATE on the tensor engine (numer
    # matmuls and the D chain), so the PE never drains waiting on the
    # trailing exps of the current block (a drained PE gets clock-gated by
    # the HAM and runs at half rate for ~3us after).
    msks = {}
    nmr = PS_A.tile([128, 32], F32, name="nmr", tag="acc")  # col = a*8+c

    def numer_mm(a, c):
        nc.tensor.matmul(nmr[:, a * 8 + c:a * 8 + c + 1],
                         lhsT=msks.pop((a, c))[:],
                         rhs=ones_c[:], start=True, stop=True,
                         skip_group_check=True)

    def d_chain(a):
        # D = numer * DS/Z -> transpose to rows 0..7 -> selector-matmul
        # broadcast (all reads at base partition 0; offset bases hang the HW)
        ts = slice(a * 128, (a + 1) * 128)
        zrt = SCR.tile([128, 8], F32, name="zrt", tag="zrt", bufs=2)
        nc.vector.reciprocal(zrt[:], Zt[:, a * 8:a * 8 + 8])
        nc.vector.tensor_scalar_mul(zrt[:], zrt[:], DS)
        dc = SCR.tile([128, 8], BF16, name="dc", tag="dc", bufs=2)
        nc.vector.tensor_mul(dc[:], nmr[:, a * 8:a * 8 + 8], zrt[:])
        dt_ps = PS_A.tile([128, 128], BF16, name="dt_ps", tag="acc")
        nc.tensor.transpose(dt_ps[0:8, :], dc[:], identb[:])
        dt_sb = SCR.tile([128, 128], BF16, name="dt_sb", tag="dts", bufs=2)
        nc.vector.tensor_copy(dt_sb[0:8, :], dt_ps[0:8, :])
        bc_ps = PS_S.tile([128, 1024], F32, name="bc_ps", tag="S")
        for c in range(8):
            nc.tensor.matmul(bc_ps[:, c * 128:(c + 1) * 128],
                             lhsT=sel8[0:8, c * 128:(c + 1) * 128],
                             rhs=dt_sb[0:8, :],
                             start=True, stop=True, skip_group_check=True)
        nc.vector.tensor_copy(Bc3[:, :, ts],
                              bc_ps.rearrange("p (c t) -> p c t", c=8))

    # Merged Q-projection + attention: Q chunks for column-group c1 are
    # produced right before the four S groups that consume them, so the
    # exp stream starts ~30us earlier than a separate Q phase would allow.
    wqsrc = d["wq"].rearrange("(t p) f -> t p f", p=128)
    wqt = {}
    for q in range(4):
        for pp in range(2):
            w = PW.tile([128, 2048], F8, name=f"w_wq{q}{pp}", tag="w")
            nc.sync.dma_start(w[:], wqsrc[pp * 4 + q])
            wqt[(q, pp)] = w
    pend = []
    for c1 in range(8):
        for k in range(4):
            m = 4 * c1 + k
            q, mq = m // 8, m % 8
            psq = PS_A.tile([128, 512], F32, name="psq", tag="acc")
            for pp in range(2):
                lhsT = wqt[(q, pp)].rearrange(
                    "p (j m) -> p j m", j=2)[:, :, mq * 128:(mq + 1) * 128]
                nc.tensor.matmul(psq[:], lhsT=lhsT, rhs=xTp[pp],
                                 start=(pp == 0), stop=(pp == 1),
                                 perf_mode=DR)
            nc.vector.tensor_copy(QT[:, m * 512:(m + 1) * 512], psq[:])
        for a in range(NBLK):
            ts = slice(a * 128, (a + 1) * 128)
            ps = PS_S.tile([128, 1024], F32, name="ps_s", tag="S")
            for half in range(2):
                for pp in range(2):
                    lhsT = QT4[:, c1, 2 * pp:2 * pp + 2, ts]
                    rhs = KT4[:, 4 * half:4 * half + 4,
                              2 * pp:2 * pp + 2, ts].transpose([0, 2, 1, 3])
                    nc.tensor.matmul(
                        ps[:, half * 512:(half + 1) * 512],
                        lhsT=lhsT, rhs=rhs,
                        start=(pp == 0), stop=(pp == 1), perf_mode=DR)
            if len(pend) >= 8:
                pend.pop(0)()
            junk = SCR.tile([128, 1024], BF16, name="junk", tag="junk", bufs=2)
            nc.scalar.activation(junk[:], ps[:], AF.Exp, scale=ESC,
                                 accum_out=Zt[:, a * 8 + c1:a * 8 + c1 + 1])
            msk = SCR.tile([128, 128], BF16, name="msk", tag="msk", bufs=9)
            nc.vector.tensor_mul(msk[:], junk[:, c1 * 128:(c1 + 1) * 128],
                                 identb[:])
            msks[(a, c1)] = msk
            pend.append(lambda a=a, c1=c1: numer_mm(a, c1))
    for f in pend:
        f()
    for a in range(NBLK):
        d_chain(a)

    # ---- gamma/beta broadcast rows -> gb
    for i in range(4):
        psg = PS_A.tile([128, 512], F32, name="psg", tag="acc")
        nc.tensor.matmul(psg[:], lhsT=ones128[0:1, :],
                         rhs=vrow[0:1, i * 512:(i + 1) * 512],
                         start=True, stop=True, tile_position=(0, 0))
        nc.scalar.copy(gb[:, i * 512:(i + 1) * 512], psg[:])

    # ---- V projection fused with diag-softmax scaling -> hhT,
    # then attn out = hhT @ Wo ; + residual ; LN1 core.
    # Chunk pairs share one 2-bank psum so the evac-mult runs at [128,1024]
    # granularity (halves the DVE per-op overhead).
    hhT3 = hhT.rearrange("p (m t) -> p m t", t=512)
    wvsrc = d["wv"].rearrange("(t p) f -> t p f", p=128)
    wvt = {}
    for q in range(4):
        for pp in range(2):
            w = PW.tile([128, 2048], F8, name=f"w_wv{q}{pp}", tag="w")
            nc.sync.dma_start(w[:], wvsrc[pp * 4 + q])
            wvt[(q, pp)] = w
    for mp in range(16):
        ps = PS_S.tile([128, 1024], F32, name="ps_v", tag="S")
        for k in range(2):
            m = 2 * mp + k
            q, mq = m // 8, m % 8
            for pp in range(2):
                lhsT = wvt[(q, pp)].rearrange(
                    "p (j m) -> p j m", j=2)[:, :, mq * 128:(mq + 1) * 128]
                nc.tensor.matmul(ps[:, k * 512:(k + 1) * 512], lhsT=lhsT,
                                 rhs=xTp[pp], start=(pp == 0), stop=(pp == 1),
                                 perf_mode=DR)
        bsl = Bc3[:, mp // 2:mp // 2 + 1, :].broadcast_to((128, 2, 512))
        nc.vector.tensor_mul(
            hhT[:, mp * 1024:(mp + 1) * 1024].rearrange(
                "p (k t) -> p k t", k=2),
            ps.rearrange("p (k t) -> p k t", k=2), bsl)

    def ln_core(v_ap, out_ap):
        # mean via scalar Copy+accum (keeps the DVE free for the evac mults)
        nmu = ST.tile([128, 1], F32, name="nmu", tag="nmu")
        junkm = SCR.tile([128, 512], BF16, name="junkm", tag="junkf", bufs=2)
        nc.scalar.activation(junkm[:], v_ap, AF.Copy, accum_out=nmu[:])
        nc.vector.tensor_scalar_mul(nmu[:], nmu[:], -1.0 / H)
        ssq = ST.tile([128, 1], F32, name="ssq", tag="ssq")
        junkf = SCR.tile([128, 512], BF16, name="junkf", tag="junkf", bufs=2)
        nc.scalar.activation(junkf[:], v_ap, AF.Square, bias=nmu[:],
                             accum_out=ssq[:])
        sd = ST.tile([128, 1], F32, name="sd", tag="sd")
        nc.scalar.activation(sd[:], ssq[:], AF.Sqrt, scale=1.0 / H,
                             bias=epsc[:])
        rs = ST.tile([128, 1], F32, name="rs", tag="rs")
        nc.vector.reciprocal(rs[:], sd[:])
        nc.vector.tensor_scalar(out_ap, v_ap, nmu[:], rs[:],
                                op0=ALU.add, op1=ALU.mult)

    def ln(v_ap, gofs, bofs, out_ap):
        ln_core(v_ap, out_ap)
        nc.vector.tensor_mul(out_ap, out_ap, gb[:, gofs * 512:(gofs + 1) * 512])
        nc.vector.tensor_add(out_ap, out_ap, gb[:, bofs * 512:(bofs + 1) * 512])

    # Wo chains interleaved with the xcr->hh1T transposes (fp32, for FFN1)
    # so the PE never drains waiting on a trailing LN1 chain.
    wos3 = wos.rearrange("p (i j h) -> p i j h", i=16, j=2)

    def wo_chain(mt):
        ps_o = PS_A.tile([128, 512], F32, name="ps_o", tag="acc")
        for i in range(16):
            nc.tensor.matmul(
                ps_o[:], lhsT=hhT3[:, 2 * i:2 * i + 2, mt * 128:(mt + 1) * 128],
                rhs=wos3[:, i], start=(i == 0), stop=(i == 15), perf_mode=DR)
        v1 = SCR.tile([128, 512], F32, name="v1", tag="xr")
        nc.vector.scalar_tensor_tensor(
            out=v1[:], in0=ps_o[:], scalar=ODESC,
            in1=xrs[:, mt * 512:(mt + 1) * 512], op0=ALU.mult, op1=ALU.add)
        ln_core(v1[:], xcr[:, mt * 512:(mt + 1) * 512])

    def xcr_transpose(mt):
        for jj in range(4):
            tp = PS_A.tile([128, 128], F32, name="tp_h", tag="acc")
            nc.tensor.transpose(
                tp[:], xcr[:, mt * 512 + jj * 128:mt * 512 + jj * 128 + 128],
                identf[:])
            dst = hh1T[:, jj * 512 + mt * 128:jj * 512 + mt * 128 + 128]
            if jj % 2 == 0:
                nc.vector.tensor_copy(dst, tp[:])
            else:
                nc.scalar.copy(dst, tp[:])

    wo_chain(0)
    wo_chain(1)
    xcr_transpose(0)
    wo_chain(2)
    xcr_transpose(1)
    wo_chain(3)
    xcr_transpose(2)
    xcr_transpose(3)

    # ---- FFN1 (bf16): a1T = relu(Wf1'^T @ hh1T + bf1')
    for mf in range(16):
        ps = PS_A.tile([128, 512], F32, name="ps_f1", tag="acc")
        for hc in range(4):
            nc.tensor.matmul(
                ps[:], lhsT=wf1s[:, hc * 2048 + mf * 128:hc * 2048 + mf * 128 + 128],
                rhs=hh1T[:, hc * 512:(hc + 1) * 512],
                start=(hc == 0), stop=(hc == 3))
        nc.scalar.activation(a1T[:, mf * 512:(mf + 1) * 512], ps[:], AF.Relu,
                             bias=bf1c[:, mf:mf + 1])

    # hh1r = xcr*g1 + (beta1+bf2): emitted here so it lands in the FFN1
    # window where the DVE is otherwise idle (it's only read by FFN2's s2)
    for mt in range(4):
        sl = slice(mt * 512, (mt + 1) * 512)
        nc.vector.tensor_mul(hh1r[:, sl], xcr[:, sl], gb[:, 0:512])
        nc.vector.tensor_add(hh1r[:, sl], hh1r[:, sl], gb[:, 512:1024])

    # ---- FFN2 (bf16) + residual + LN2 -> out
    for mt in range(4):
        ps = PS_A.tile([128, 512], F32, name="ps_f2", tag="acc")
        for fc in range(16):
            nc.tensor.matmul(
                ps[:],
                lhsT=a1T[:, fc * 512 + mt * 128:fc * 512 + mt * 128 + 128],
                rhs=wf2s[:, fc * 512:(fc + 1) * 512],
                start=(fc == 0), stop=(fc == 15))
        s2 = SCR.tile([128, 512], F32, name="s2", tag="xr")
        nc.vector.tensor_add(s2[:], ps[:], hh1r[:, mt * 512:(mt + 1) * 512])
        outt = SCR.tile([128, 512], F32, name="outt", tag="xr")
        ln(s2[:], 2, 3, outt[:])
        nc.sync.dma_start(d["out"][mt * 128:(mt + 1) * 128, :], outt[:])

    for pool in (PS_B, PS_A, PS_S, ST, SCR, PW, P):
        pool.release()


def build(loop_n=None):
    nc = bacc.Bacc("TRN2", target_bir_lowering=False)
    d = {
        "xT": nc.dram_tensor("xT", (TPC, H), F8, kind="ExternalInput").ap(),
        "xr": nc.dram_tensor("xr", (TPC, H), F32, kind="ExternalInput").ap(),
        "wq": nc.dram_tensor("wq", (1024, 2048), F8, kind="ExternalInput").ap(),
        "wk": nc.dram_tensor("wk", (1024, 2048), F8, kind="ExternalInput").ap(),
        "wv": nc.dram_tensor("wv", (1024, 2048), F8, kind="ExternalInput").ap(),
        "wo": nc.dram_tensor("wo", (2048, 1024), F8, kind="ExternalInput").ap(),
        "wf1": nc.dram_tensor("wf1", (H, 4 * H), BF16, kind="ExternalInput").ap(),
        "wf2": nc.dram_tensor("wf2", (4 * H, H), BF16, kind="ExternalInput").ap(),
        "bf1": nc.dram_tensor("bf1", (4 * H,), F32, kind="ExternalInput").ap(),
        "vecs": nc.dram_tensor("vecs", (4, H), BF16, kind="ExternalInput").ap(),
        "ident": nc.dram_tensor("ident", (128, 128), F32,
                                kind="ExternalInput").ap(),
        "sel8": nc.dram_tensor("sel8", (8, 8 * 128), BF16,
                               kind="ExternalInput").ap(),
        "out": nc.dram_tensor("out", (TPC, H), F32, kind="ExternalOutput").ap(),
    }
    with tile.TileContext(nc) as tc:
        if loop_n is None:
            _emit(nc, tc, d)
        else:
            with tc.For_i(0, loop_n, 1):
                _emit(nc, tc, d)
    nc.finalize()
    return nc


def _pack_w(W):
    # tile-major fp8 packing for DoubleRow projections:
    # tile (pair, q): [p, j*1024+m] = W[(2*pair+j)*128+p, q*1024+m]
    W5 = np.asarray(W, np.float32).reshape(2, 2, 128, 4, 1024)
    return np.ascontiguousarray(
        W5.transpose(0, 3, 2, 1, 4).reshape(8 * 128, 2048)).astype(_F8)


def _pack_wo(W):
    # tile i: [p, j*512+h] = Wo[(2*i+j)*128+p, h]
    W4 = np.asarray(W, np.float32).reshape(16, 2, 128, 512)
    return np.ascontiguousarray(
        W4.transpose(0, 2, 1, 3).reshape(16 * 128, 1024)).astype(_F8)


def make_in_maps(inputs):
    xf = np.ascontiguousarray(
        np.asarray(inputs["x"], np.float32).reshape(TOK, H))
    g1 = np.asarray(inputs["g1"], np.float32)
    wf1 = np.asarray(inputs["Wf1"], np.float32)
    shared = {
        "wq": _pack_w(WS * np.asarray(inputs["Wq"], np.float32)),
        "wk": _pack_w(WS * np.asarray(inputs["Wk"], np.float32)),
        "wv": _pack_w(WS * np.asarray(inputs["Wv"], np.float32)),
        "wo": _pack_wo(WS * np.asarray(inputs["Wo"], np.float32)),
        "wf1": (g1[:, None] * wf1).astype(_BF),
        "wf2": np.asarray(inputs["Wf2"], np.float32).astype(_BF),
        "bf1": (np.asarray(inputs["bf1"], np.float32)
                + np.asarray(inputs["beta1"], np.float32) @ wf1),
        "vecs": np.ascontiguousarray(np.stack([
            g1,
            np.asarray(inputs["beta1"], np.float32)
            + np.asarray(inputs["bf2"], np.float32),
            np.asarray(inputs["g2"], np.float32),
            np.asarray(inputs["beta2"], np.float32)]).astype(_BF)),
        "ident": np.eye(128, dtype=np.float32),
        "sel8": np.ascontiguousarray(
            np.kron(np.eye(8, dtype=np.float32), np.ones((1, 128), np.float32))
            .astype(_BF)),
    }
    in_maps = []
    for c in range(NCORES):
        xs = xf[c * TPC:(c + 1) * TPC]
        m = dict(shared)
        m["xT"] = np.ascontiguousarray(xs.T).astype(_F8)
        m["xr"] = np.ascontiguousarray(xs)
        in_maps.append(m)
    return in_maps


_nc_cache = None


def _get_nc():
    global _nc_cache
    if _nc_cache is None:
        _nc_cache = build()
    return _nc_cache


def kernel(**inputs):
    nc = _get_nc()
    in_maps = make_in_maps(inputs)
    res = bass_utils.run_bass_kernel_spmd(nc, in_maps,
                                          core_ids=list(range(NCORES)))
    out = np.concatenate([r["out"] for r in res.results], axis=0)
    return out.reshape(B, T, H)


if __name__ == "__main__":
    nc = build()
    n_inst = sum(len(bb.instructions) for bb in nc.main_func.blocks)
    print("built OK; instructions:", n_inst)


# revision 40
# speedup vs baseline: 1.3078x; 1.0500x over previous
"""Trainium2 Bass kernel for the MultiHeadAttention-variant transformer block.

Math notes (derived from the module semantics):
  - The einsum 'batt,bath->bath' uses only the DIAGONAL of the softmax'd
    attention matrix, so per flat row i the attention output is
    softmax_diag_i * V[i].
  - The raw reshape (B,T,N*H)->(B,N,T,H) makes attention "blocks" couple only
    groups of 128 consecutive tokens (T/N = 1024/8 = 128); a block's 1024
    logical rows are the (chunk c, token t) pairs of those 128 tokens.
  => The whole layer is data-parallel over 128-token groups. We shard the
     4096 flattened tokens as 512 consecutive tokens per core (8 cores), with
     zero cross-core communication.

v2 design (vs the v1 baseline):
  - S is computed Q-STATIONARY with rows enumerated r = c*128 + t, so each
    row-tile of the 1024x1024 block-attention matrix is one out-psum
    [128, 1024] whose FREE axis spans the full softmax denominator.  The
    row sums Z then come for free from the Exp activation's accum_out, and
    the numerators are the diagonals of the c-th 128-col group (identity
    mask * ones-column matmul).  This removes the v1 Z-row matmul chains,
    the [1,512] reciprocals (28us of DVE!), and most broadcast plumbing.
  - QKV and Wo matmuls run fp8e4m3 DoubleRow (the FFN stays bf16: fp8
    there costs ~1.4e-2 rel err).  Weights are pre-scaled by 16 on the
    host (values ~0.02*N(0,1) would otherwise sit at the bottom of the
    fp8 range); the scales are folded into the exp() activation scale
    and the residual descale.
  - K-proj runs first, then Q-proj chunks are interleaved with the S
    matmul groups so the (scalar-bound) exp stream starts early; numer
    matmuls and D chains are emitted one block late so the PE never
    drains (a drained PE gets HAM clock-gated to half rate).
  - LDWEIGHTS is fully hidden behind 512-col DR matmuls (216ns cadence),
    so the tensor-engine floor is the streamed column count.
"""

import sys

sys.path.insert(0, "/opt/trn_rl_repo")

import numpy as np
import ml_dtypes

import concourse.bass as bass
import concourse.mybir as mybir
import concourse.tile as tile
from concourse import bacc, bass_utils

F32 = mybir.dt.float32
BF16 = mybir.dt.bfloat16
F8 = mybir.dt.float8e4
AF = mybir.ActivationFunctionType
ALU = mybir.AluOpType
AX = mybir.AxisListType
DR = mybir.MatmulPerfMode.DoubleRow

H = 512
NH = 8
B = 4
T = 1024
TOK = B * T
NCORES = 8
TPC = TOK // NCORES  # 512 tokens per core
NBLK = TPC // 128  # 4 attention blocks per core
SCALE = float(1.0 / np.sqrt(H))
LN_EPS = 1e-5

_BF = ml_dtypes.bfloat16
_F8 = ml_dtypes.float8_e4m3

WS = 16.0  # host pre-scale on every weight matrix (fp8 range)
DS = 64.0  # extra scale on the softmax-diag D (fp8 range of hhT)
ESC = SCALE / (WS * WS)  # exp() input scale: undo Wq*16 * Wk*16
ODESC = 1.0 / (WS * WS * DS)  # Wo-path descale: V*16, Wo*16, D*64
FDESC = 1.0 / WS  # FFN descales


def _emit(nc, tc, d):
    """Emit the per-core program. d: dict of DRAM APs."""
    P = tc.alloc_tile_pool(name="persist", bufs=1)
    PW = tc.alloc_tile_pool(name="wpool", bufs=16)
    SCR = tc.alloc_tile_pool(name="scr", bufs=4)
    ST = tc.alloc_tile_pool(name="stats", bufs=4)
    PS_S = tc.alloc_tile_pool(name="ps_s", bufs=2, space="PSUM")
    PS_A = tc.alloc_tile_pool(name="ps_a", bufs=4, space="PSUM")

    # ---- persistent tiles
    xT = P.tile([128, 4 * 512], F8, name="xT")  # x^T, 4 h-chunks
    identf = P.tile([128, 128], F32, name="identf")
    identb = P.tile([128, 128], BF16, name="identb")
    ones_c = P.tile([128, 1], BF16, name="ones_c")  # rhs for diag-col matmul
    ones128 = P.tile([128, 128], BF16, name="ones128")  # bcast lhsT rows
    vrow = P.tile([1, 4 * 512], BF16, name="vrow")  # g1,vsum,g2,b2 rows
    bf1c = P.tile([128, 16], F32, name="bf1c")  # bf1 column-major
    gb = P.tile([128, 4 * 512], BF16, name="gb")  # bcast g1,vsum,g2,b2
    Bc = P.tile([128, 8 * 512], F8, name="Bc")  # D*DS bcast, [p, c*512+t]
    Zt = P.tile([128, 32], F32, name="Zt")  # softmax denoms, col a*8+c
    epsc = P.tile([128, 1], F32, name="epsc")
    xrs = P.tile([128, 4 * 512], F32, name="xrs")  # residual x, [p, mt*512+h]
    sel8 = P.tile([128, 8 * 128], BF16, name="sel8")  # sel8[k,c*128+p]=(k==c)
    QT = P.tile([128, 32 * 512], F8, name="QT")
    KT = P.tile([128, 32 * 512], F8, name="KT")
    hhT = P.tile([128, 32 * 512], F8, name="hhT")
    xcr = P.tile([128, 4 * 512], F32, name="xcr")  # LN1 core out (no g/b)
    hh1T = P.tile([128, 4 * 512], BF16, name="hh1T")  # xcr^T for FFN1
    hh1r = P.tile([128, 4 * 512], F32, name="hh1r")  # true hh1 residual
    a1T = P.tile([128, 16 * 512], BF16, name="a1T")  # relu acts, [p, fc*512+t]
    wos = P.tile([128, 16 * 1024], F8, name="wos")  # Wo DR-packed
    wf1s = P.tile([128, 4 * 2048], BF16, name="wf1s")  # [p, hc*2048+f]
    wf2s = P.tile([128, 16 * 512], BF16, name="wf2s")  # [p, fc*512+h]

    # ---- input DMAs.  sync ring: critical path (x, ident, wq/wk/wv stream).
    # scalar ring: everything needed later (vecs, bf1, xr, wo, wf1, wf2).
    nc.sync.dma_start(
        xT.rearrange("p (hc t) -> p hc t", hc=4),
        d["xT"].rearrange("(hc p) t -> p hc t", p=128))
    nc.sync.dma_start(identf[:], d["ident"][:])
    nc.scalar.dma_start(vrow[0:1, :], d["vecs"].rearrange("v h -> (v h)")[None, :])
    nc.scalar.dma_start(bf1c[:], d["bf1"].rearrange("(m p) -> p m", p=128))

    nc.vector.memset(ones_c[:], 1.0)
    nc.vector.memset(ones128[:], 1.0)
    nc.vector.memset(epsc[:], LN_EPS)
    nc.vector.tensor_copy(identb[:], identf[:])
    # dummy exp: pulls the exp ACT_TABLE_LOAD (~1.3us) off the first real
    # exp's critical path (the scheduler hoists this to the idle head)
    dume = ST.tile([128, 1], F32, name="dume", tag="nmu")
    nc.scalar.activation(dume[:], epsc[:], AF.Exp)

    def late_dmas():
        # big late-deadline weights go on the (otherwise idle) gpsimd SWDGE
        # ring, gated behind the QK weight stream (emitted mid-phase-A) so
        # their 7MB doesn't steal fabric bandwidth from the critical path
        nc.gpsimd.dma_start(sel8[0:8, :], d["sel8"][:])
        nc.gpsimd.dma_start(
            xrs.rearrange("p (mt h) -> p mt h", mt=4),
            d["xr"].rearrange("(mt p) h -> p mt h", p=128))
        nc.gpsimd.dma_start(
            wos.rearrange("p (i f) -> p i f", i=16),
            d["wo"].rearrange("(i p) f -> p i f", p=128))
        nc.gpsimd.dma_start(
            wf1s.rearrange("p (hc f) -> p hc f", hc=4),
            d["wf1"].rearrange("(hc p) f -> p hc f", p=128))
        nc.gpsimd.dma_start(
            wf2s.rearrange("p (fc h) -> p fc h", fc=16),
            d["wf2"].rearrange("(fc p) h -> p fc h", p=128))

    # ---- projections (fp8 DoubleRow): dst^T = W^T @ x^T.  Weight DRAM is
    # pre-packed tile-major: tile (pair, q) rows,
    # [p, j*1024+m] = W[(2*pair+j)*128+p, q*1024+m].
    xTp = [xT[:, pp * 1024:(pp + 1) * 1024].rearrange("p (j t) -> p j t", j=2)
           for pp in range(2)]

    def proj(wname, evac):
        wsrc = d[wname].rearrange("(t p) f -> t p f", p=128)
        wt = {}
        for q in range(4):
            for pp in range(2):
                w = PW.tile([128, 2048], F8, name=f"w_{wname}{q}{pp}", tag="w")
                nc.sync.dma_start(w[:], wsrc[pp * 4 + q])
                wt[(q, pp)] = w
        for m in range(32):
            q, mq = m // 8, m % 8
            ps = PS_A.tile([128, 512], F32, name="ps_proj", tag="acc")
            for pp in range(2):
                lhsT = wt[(q, pp)].rearrange(
                    "p (j m) -> p j m", j=2)[:, :, mq * 128:(mq + 1) * 128]
                nc.tensor.matmul(ps[:], lhsT=lhsT, rhs=xTp[pp],
                                 start=(pp == 0), stop=(pp == 1),
                                 perf_mode=DR)
            evac(m, ps)

    def evac_alt(dst):
        def f(m, ps):
            sl = dst[:, m * 512:(m + 1) * 512]
            if m % 2 == 0:
                nc.vector.tensor_copy(sl, ps[:])
            else:
                nc.scalar.copy(sl, ps[:])
        return f

    proj("wk", evac_alt(KT))
    # data-dependency gates: the scheduler reorders engine instructions, so
    # each bulk prefetch needs a real WAW dependency — a tiny copy that
    # reads the end of KT and writes into the DMA's destination tile.
    for gt in (sel8, xrs, wos, wf1s, wf2s):
        nc.vector.tensor_copy(gt[0:1, 0:1], KT[0:1, 16383:16384])
    late_dmas()

    # ---- attention: Q-stationary S tiles, Z via exp-accum, diag numerators.
    # Row/col enumeration within a block: r = c*128 + t_local.
    QT4 = QT.rearrange("p (c hc t) -> p c hc t", c=8, hc=4)
    KT4 = KT.rearrange("p (c hc t) -> p c hc t", c=8, hc=4)
    Bc3 = Bc.rearrange("p (c t) -> p c t", c=8)

    # Per-block work is emitted one block LATE on the tensor engine (numer
    # matmuls and the D chain), so the PE never drains waiting on the
    # trailing exps of the current block (a drained PE gets clock-gated by
    # the HAM and runs at half rate for ~3us after).
    msks = {}
    nmr = PS_A.tile([128, 32], F32, name="nmr", tag="acc")  # col = a*8+c

    def numer_mm(a, c):
        nc.tensor.matmul(nmr[:, a * 8 + c:a * 8 + c + 1],
                         lhsT=msks.pop((a, c))[:],
                         rhs=ones_c[:], start=True, stop=True,
                         skip_group_check=True)

    def d_chain(a):
        # D = numer * DS/Z -> transpose to rows 0..7 -> selector-matmul
        # broadcast (all reads at base partition 0; offset bases hang the HW)
        ts = slice(a * 128, (a + 1) * 128)
        zrt = SCR.tile([128, 8], F32, name="zrt", tag="zrt", bufs=2)
        nc.vector.reciprocal(zrt[:], Zt[:, a * 8:a * 8 + 8])
        nc.vector.tensor_scalar_mul(zrt[:], zrt[:], DS)
        dc = SCR.tile([128, 8], BF16, name="dc", tag="dc", bufs=2)
        nc.vector.tensor_mul(dc[:], nmr[:, a * 8:a * 8 + 8], zrt[:])
        dt_ps = PS_A.tile([128, 128], BF16, name="dt_ps", tag="acc")
        nc.tensor.transpose(dt_ps[0:8, :], dc[:], identb[:])
        dt_sb = SCR.tile([128, 128], BF16, name="dt_sb", tag="dts", bufs=2)
        nc.vector.tensor_copy(dt_sb[0:8, :], dt_ps[0:8, :])
        bc_ps = PS_S.tile([128, 1024], F32, name="bc_ps", tag="S")
        for c in range(8):
            nc.tensor.matmul(bc_ps[:, c * 128:(c + 1) * 128],
                             lhsT=sel8[0:8, c * 128:(c + 1) * 128],
                             rhs=dt_sb[0:8, :],
                             start=True, stop=True, skip_group_check=True)
        nc.vector.tensor_copy(Bc3[:, :, ts],
                              bc_ps.rearrange("p (c t) -> p c t", c=8))

    # Merged Q-projection + attention: Q chunks for column-group c1 are
    # produced right before the four S groups that consume them, so the
    # exp stream starts ~30us earlier than a separate Q phase would allow.
    wqsrc = d["wq"].rearrange("(t p) f -> t p f", p=128)
    wqt = {}
    for q in range(4):
        for pp in range(2):
            w = PW.tile([128, 2048], F8, name=f"w_wq{q}{pp}", tag="w")
            nc.sync.dma_start(w[:], wqsrc[pp * 4 + q])
            wqt[(q, pp)] = w
    pend = []
    for c1 in range(8):
        for k in range(4):
            m = 4 * c1 + k
            q, mq = m // 8, m % 8
            psq = PS_A.tile([128, 512], F32, name="psq", tag="acc")
            for pp in range(2):
                lhsT = wqt[(q, pp)].rearrange(
                    "p (j m) -> p j m", j=2)[:, :, mq * 128:(mq + 1) * 128]
                nc.tensor.matmul(psq[:], lhsT=lhsT, rhs=xTp[pp],
                                 start=(pp == 0), stop=(pp == 1),
                                 perf_mode=DR)
            nc.vector.tensor_copy(QT[:, m * 512:(m + 1) * 512], psq[:])
        for a in range(NBLK):
            ts = slice(a * 128, (a + 1) * 128)
            ps = PS_S.tile([128, 1024], F32, name="ps_s", tag="S")
            for half in range(2):
                for pp in range(2):
                    lhsT = QT4[:, c1, 2 * pp:2 * pp + 2, ts]
                    rhs = KT4[:, 4 * half:4 * half + 4,
                              2 * pp:2 * pp + 2, ts].transpose([0, 2, 1, 3])
                    nc.tensor.matmul(
                        ps[:, half * 512:(half + 1) * 512],
                        lhsT=lhsT, rhs=rhs,
                        start=(pp == 0), stop=(pp == 1), perf_mode=DR)
            if len(pend) >= 8:
                pend.pop(0)()
            junk = SCR.tile([128, 1024], BF16, name="junk", tag="junk", bufs=2)
            nc.scalar.activation(junk[:], ps[:], AF.Exp, scale=ESC,
                                 accum_out=Zt[:, a * 8 + c1:a * 8 + c1 + 1])
            msk = SCR.tile([128, 128], BF16, name="msk", tag="msk", bufs=9)
            nc.vector.tensor_mul(msk[:], junk[:, c1 * 128:(c1 + 1) * 128],
                                 identb[:])
            msks[(a, c1)] = msk
            pend.append(lambda a=a, c1=c1: numer_mm(a, c1))
    for f in pend:
        f()
    for a in range(NBLK):
        d_chain(a)

    # ---- gamma/beta broadcast rows -> gb
    for i in range(4):
        psg = PS_A.tile([128, 512], F32, name="psg", tag="acc")
        nc.tensor.matmul(psg[:], lhsT=ones128[0:1, :],
                         rhs=vrow[0:1, i * 512:(i + 1) * 512],
                         start=True, stop=True, tile_position=(0, 0))
        nc.scalar.copy(gb[:, i * 512:(i + 1) * 512], psg[:])

    # ---- V projection fused with diag-softmax scaling -> hhT,
    # then attn out = hhT @ Wo ; + residual ; LN1 core.
    # Chunk pairs share one 2-bank psum so the evac-mult runs at [128,1024]
    # granularity (halves the DVE per-op overhead).
    hhT3 = hhT.rearrange("p (m t) -> p m t", t=512)
    wvsrc = d["wv"].rearrange("(t p) f -> t p f", p=128)
    wvt = {}
    for q in range(4):
        for pp in range(2):
            w = PW.tile([128, 2048], F8, name=f"w_wv{q}{pp}", tag="w")
            nc.sync.dma_start(w[:], wvsrc[pp * 4 + q])
            wvt[(q, pp)] = w
    for mp in range(16):
        ps = PS_S.tile([128, 1024], F32, name="ps_v", tag="S")
        for k in range(2):
            m = 2 * mp + k
            q, mq = m // 8, m % 8
            for pp in range(2):
                lhsT = wvt[(q, pp)].rearrange(
                    "p (j m) -> p j m", j=2)[:, :, mq * 128:(mq + 1) * 128]
                nc.tensor.matmul(ps[:, k * 512:(k + 1) * 512], lhsT=lhsT,
                                 rhs=xTp[pp], start=(pp == 0), stop=(pp == 1),
                                 perf_mode=DR)
        bsl = Bc3[:, mp // 2:mp // 2 + 1, :].broadcast_to((128, 2, 512))
        nc.vector.tensor_mul(
            hhT[:, mp * 1024:(mp + 1) * 1024].rearrange(
                "p (k t) -> p k t", k=2),
            ps.rearrange("p (k t) -> p k t", k=2), bsl)

    def ln_core(v_ap, out_ap):
        # mean via scalar Copy+accum (keeps the DVE free for the evac mults)
        nmu = ST.tile([128, 1], F32, name="nmu", tag="nmu")
        junkm = SCR.tile([128, 512], BF16, name="junkm", tag="junkf", bufs=2)
        nc.scalar.activation(junkm[:], v_ap, AF.Copy, accum_out=nmu[:])
        nc.vector.tensor_scalar_mul(nmu[:], nmu[:], -1.0 / H)
        ssq = ST.tile([128, 1], F32, name="ssq", tag="ssq")
        junkf = SCR.tile([128, 512], BF16, name="junkf", tag="junkf", bufs=2)
        nc.scalar.activation(junkf[:], v_ap, AF.Square, bias=nmu[:],
                             accum_out=ssq[:])
        sd = ST.tile([128, 1], F32, name="sd", tag="sd")
        nc.scalar.activation(sd[:], ssq[:], AF.Sqrt, scale=1.0 / H,
                             bias=epsc[:])
        rs = ST.tile([128, 1], F32, name="rs", tag="rs")
        nc.vector.reciprocal(rs[:], sd[:])
        nc.vector.tensor_scalar(out_ap, v_ap, nmu[:], rs[:],
                                op0=ALU.add, op1=ALU.mult)

    def ln(v_ap, gofs, bofs, out_ap):
        ln_core(v_ap, out_ap)
        nc.vector.tensor_mul(out_ap, out_ap, gb[:, gofs * 512:(gofs + 1) * 512])
        nc.vector.tensor_add(out_ap, out_ap, gb[:, bofs * 512:(bofs + 1) * 512])

    # Wo chains interleaved with the xcr->hh1T transposes (fp32, for FFN1)
    # so the PE never drains waiting on a trailing LN1 chain.
    wos3 = wos.rearrange("p (i j h) -> p i j h", i=16, j=2)

    def wo_chain(mt):
        ps_o = PS_A.tile([128, 512], F32, name="ps_o", tag="acc")
        for i in range(16):
            nc.tensor.matmul(
                ps_o[:], lhsT=hhT3[:, 2 * i:2 * i + 2, mt * 128:(mt + 1) * 128],
                rhs=wos3[:, i], start=(i == 0), stop=(i == 15), perf_mode=DR)
        v1 = SCR.tile([128, 512], F32, name="v1", tag="xr")
        nc.vector.scalar_tensor_tensor(
            out=v1[:], in0=ps_o[:], scalar=ODESC,
            in1=xrs[:, mt * 512:(mt + 1) * 512], op0=ALU.mult, op1=ALU.add)
        ln_core(v1[:], xcr[:, mt * 512:(mt + 1) * 512])

    def xcr_transpose(mt):
        for jj in range(4):
            tp = PS_A.tile([128, 128], F32, name="tp_h", tag="acc")
            nc.tensor.transpose(
                tp[:], xcr[:, mt * 512 + jj * 128:mt * 512 + jj * 128 + 128],
                identf[:])
            dst = hh1T[:, jj * 512 + mt * 128:jj * 512 + mt * 128 + 128]
            if jj % 2 == 0:
                nc.vector.tensor_copy(dst, tp[:])
            else:
                nc.scalar.copy(dst, tp[:])

    wo_chain(0)
    wo_chain(1)
    xcr_transpose(0)
    wo_chain(2)
    xcr_transpose(1)
    wo_chain(3)
    xcr_transpose(2)
    xcr_transpose(3)

    # ---- FFN1 (bf16): a1T = relu(Wf1'^T @ hh1T + bf1')
    for mf in range(16):
        ps = PS_A.tile([128, 512], F32, name="ps_f1", tag="acc")
        for hc in range(4):
            nc.tensor.matmul(
                ps[:], lhsT=wf1s[:, hc * 2048 + mf * 128:hc * 2048 + mf * 128 + 128],
                rhs=hh1T[:, hc * 512:(hc + 1) * 512],
                start=(hc == 0), stop=(hc == 3))
        nc.scalar.activation(a1T[:, mf * 512:(mf + 1) * 512], ps[:], AF.Relu,
                             bias=bf1c[:, mf:mf + 1])

    # hh1r = xcr*g1 + (beta1+bf2): emitted here so it lands in the FFN1
    # window where the DVE is otherwise idle (it's only read by FFN2's s2)
    for mt in range(4):
        sl = slice(mt * 512, (mt + 1) * 512)
        nc.vector.tensor_mul(hh1r[:, sl], xcr[:, sl], gb[:, 0:512])
        nc.vector.tensor_add(hh1r[:, sl], hh1r[:, sl], gb[:, 512:1024])

    # ---- FFN2 (bf16) + residual + LN2 -> out
    for mt in range(4):
        ps = PS_A.tile([128, 512], F32, name="ps_f2", tag="acc")
        for fc in range(16):
            nc.tensor.matmul(
                ps[:],
                lhsT=a1T[:, fc * 512 + mt * 128:fc * 512 + mt * 128 + 128],
                rhs=wf2s[:, fc * 512:(fc + 1) * 512],
                start=(fc == 0), stop=(fc == 15))
        s2 = SCR.tile([128, 512], F32, name="s2", tag="xr")
        nc.vector.tensor_add(s2[:], ps[:], hh1r[:, mt * 512:(mt + 1) * 512])
        outt = SCR.tile([128, 512], F32, name="outt", tag="xr")
        ln(s2[:], 2, 3, outt[:])
        nc.sync.dma_start(d["out"][mt * 128:(mt + 1) * 128, :], outt[:])

    for pool in (PS_B, PS_A, PS_S, ST, SCR, PW, P):
        pool.release()


def build(loop_n=None):
    nc = bacc.Bacc("TRN2", target_bir_lowering=False)
    d = {
        "xT": nc.dram_tensor("xT", (TPC, H), F8, kind="ExternalInput").ap(),
        "xr": nc.dram_tensor("xr", (TPC, H), F32, kind="ExternalInput").ap(),
        "wq": nc.dram_tensor("wq", (1024, 2048), F8, kind="ExternalInput").ap(),
        "wk": nc.dram_tensor("wk", (1024, 2048), F8, kind="ExternalInput").ap(),
        "wv": nc.dram_tensor("wv", (1024, 2048), F8, kind="ExternalInput").ap(),
        "wo": nc.dram_tensor("wo", (2048, 1024), F8, kind="ExternalInput").ap(),
        "wf1": nc.dram_tensor("wf1", (H, 4 * H), BF16, kind="ExternalInput").ap(),
        "wf2": nc.dram_tensor("wf2", (4 * H, H), BF16, kind="ExternalInput").ap(),
        "bf1": nc.dram_tensor("bf1", (4 * H,), F32, kind="ExternalInput").ap(),
        "vecs": nc.dram_tensor("vecs", (4, H), BF16, kind="ExternalInput").ap(),
        "ident": nc.dram_tensor("ident", (128, 128), F32,
                                kind="ExternalInput").ap(),
        "sel8": nc.dram_tensor("sel8", (8, 8 * 128), BF16,
                               kind="ExternalInput").ap(),
        "out": nc.dram_tensor("out", (TPC, H), F32, kind="ExternalOutput").ap(),
    }
    with tile.TileContext(nc) as tc:
        if loop_n is None:
            _emit(nc, tc, d)
        else:
            with tc.For_i(0, loop_n, 1):
                _emit(nc, tc, d)
    nc.finalize()
    return nc


def _pack_w(W):
    # tile-major fp8 packing for DoubleRow projections:
    # tile (pair, q): [p, j*1024+m] = W[(2*pair+j)*128+p, q*1024+m]
    W5 = np.asarray(W, np.float32).reshape(2, 2, 128, 4, 1024)
    return np.ascontiguousarray(
        W5.transpose(0, 3, 2, 1, 4).reshape(8 * 128, 2048)).astype(_F8)


def _pack_wo(W):
    # tile i: [p, j*512+h] = Wo[(2*i+j)*128+p, h]
    W4 = np.asarray(W, np.float32).reshape(16, 2, 128, 512)
    return np.ascontiguousarray(
        W4.transpose(0, 2, 1, 3).reshape(16 * 128, 1024)).astype(_F8)


def make_in_maps(inputs):
    xf = np.ascontiguousarray(
        np.asarray(inputs["x"], np.float32).reshape(TOK, H))
    g1 = np.asarray(inputs["g1"], np.float32)
    wf1 = np.asarray(inputs["Wf1"], np.float32)
    shared = {
        "wq": _pack_w(WS * np.asarray(inputs["Wq"], np.float32)),
        "wk": _pack_w(WS * np.asarray(inputs["Wk"], np.float32)),
        "wv": _pack_w(WS * np.asarray(inputs["Wv"], np.float32)),
        "wo": _pack_wo(WS * np.asarray(inputs["Wo"], np.float32)),
        "wf1": (g1[:, None] * wf1).astype(_BF),
        "wf2": np.asarray(inputs["Wf2"], np.float32).astype(_BF),
        "bf1": (np.asarray(inputs["bf1"], np.float32)
                + np.asarray(inputs["beta1"], np.float32) @ wf1),
        "vecs": np.ascontiguousarray(np.stack([
            g1,
            np.asarray(inputs["beta1"], np.float32)
            + np.asarray(inputs["bf2"], np.float32),
            np.asarray(inputs["g2"], np.float32),
            np.asarray(inputs["beta2"], np.float32)]).astype(_BF)),
        "ident": np.eye(128, dtype=np.float32),
        "sel8": np.ascontiguousarray(
            np.kron(np.eye(8, dtype=np.float32), np.ones((1, 128), np.float32))
            .astype(_BF)),
    }
    in_maps = []
    for c in range(NCORES):
        xs = xf[c * TPC:(c + 1) * TPC]
        m = dict(shared)
        m["xT"] = np.ascontiguousarray(xs.T).astype(_F8)
        m["xr"] = np.ascontiguousarray(xs)
        in_maps.append(m)
    return in_maps


_nc_cache = None


def _get_nc():
    global _nc_cache
    if _nc_cache is None:
        _nc_cache = build()
    return _nc_cache


def kernel(**inputs):
    nc = _get_nc()
    in_maps = make_in_maps(inputs)
    res = bass_utils.run_bass_kernel_spmd(nc, in_maps,
                                          core_ids=list(range(NCORES)))
    out = np.concatenate([r["out"] for r in res.results], axis=0)
    return out.reshape(B, T, H)


if __name__ == "__main__":
    nc = build()
    n_inst = sum(len(bb.instructions) for bb in nc.main_func.blocks)
    print("built OK; instructions:", n_inst)
